# revision 24
# baseline (speedup 1.0000x reference)
"""Trainium2 Bass kernel for ARM TTT multi-head self-attention (inner-GD scan).

Math per (b, h) pair (B=16, H=12, N=4096, D=64, 16 chunks of m=256 tokens):
    A_i = k_i^T k_i ;  ct_i = k_i^T (-v_i)      (token contraction)
    grad_raw_i = A_i @ W_{i-1} + ct_i
    W_i = W_{i-1} - s * grad_raw_i,  s = 1/(m*D)
    out_i = q_i @ W_i
Pairs are fully independent -> shard B over the 8 NeuronCores (24 chains/core).

v6: v5's measured bottleneck was PE instruction CADENCE (~116ns per matmul
regardless of size: LdWeights + dispatch), 12 matmuls per chunk.  v6 packs
each head-PAIR into block-diagonal 128x128 operands -> 4 matmuls per chunk:

  1. act:  ONE fp8 DoubleRow matmul per chunk contracts all 256 tokens:
           lhsT = [k0|k1] (128t x 2j x 128), rhs = [k0|k1|v0|v1] (x 256)
           -> pac[128, 256]: A0/A1 diag blocks of cols 0:128, ct0/ct1 diag
              blocks of cols 128:256 (junk off-diag).  4 chunks per pac.
  2. cast: per t-block (4 chunks), per pair, ONE activation moves the A/ct
           diag blocks into PERSISTENT pre-zeroed block-diag bf16 tiles
           (abct) - zeros off the diag keep the chain closed in block-diag.
  3. seed: matmul(pg = Id^T @ ctbd)   [start of PSUM accumulation group]
  4. grad: matmul(pg += Abd^T @ Wbd)  [stop]
     stt (DVE, ONE op): Wbd' = -s*pg + Wbd   (off-diag stays 0: 0*s+0)
  5. out:  matmul(pout = Wbd'(lhsT) @ qt[128 dpair x 256 tok]) - both heads
           in one 256-col matmul; emitted in chunk-PAIRS, one evac per pair.

The serial W-chain round trip (PE->DVE->PE ~1us) is hidden by round-robining
chunks across a 4-group window; acts/casts lead by LAG slots; outs trail.
PSUM budget (8 banks): pac 2x2 + pg 2 + pout 2.
GpSimd CANNOT access PSUM on TRN2, so casts go to Act and evacs to DVE/Act.

Device layouts (token t = c*256 + j*128 + p):
    kv (per group):  [128(p), 16(c), 2(j), 4(k0|k1|v0|v1), 64]  fp8 (v negated)
    qt (per group):  [128(hpair*64+d), 16(c), 256(t=j*128+p)]   bf16
    out (per group): [128(hpair*64+e), 16(c), 256(t)]           bf16
    W12bd: [hg, 128, 128] f32 block-diag(W_h0, W_h1); carried chain is bf16.
"""

import os
import sys

sys.path.insert(0, "/opt/trn_rl_repo")

import numpy as np

B, H, N, D = 16, 12, 4096, 64
N_ITERS = 16
M = N // N_ITERS  # 256 tokens per chunk
NCORES = 8
NB = B // NCORES  # batches per core
HG = H // 2  # head-pair groups per batch
SCALE = 1.0 / (M * D)
WAVE = 4  # chain interleave width (groups round-robined per chunk)
LAG = 14  # slots the act/cast stream leads the chain stream
CB = 4  # chunks per t-block (pac granularity)
USE_DR = True  # fp8 DoubleRow: one act matmul per chunk (else 2, j-accum)

_CACHE = {}


def _split_excess_waits(nc):
    """walrus in this env accepts at most ONE sem wait per instruction
    (two on EventSemaphore); move excess waits onto EventSemaphore
    instructions inserted just before on the same engine."""
    import concourse.mybir as mybir

    n_ev = 0
    for f in nc.m.functions:
        for b in f.blocks:
            il = b.instructions
            idx = 0
            while idx < len(il):
                inst = il[idx]
                si = getattr(inst, "sync_info", None)
                if si is not None and len(si.on_wait) > 1:
                    waits = list(si.on_wait)
                    si.on_wait = [waits[0]]
                    extra = waits[1:]
                    for g in range(0, len(extra), 2):
                        n_ev += 1
                        ev = mybir.InstEventSemaphore(
                            name=f"EVSPLIT-{n_ev}",
                            engine=inst.engine,
                            ins=[],
                            outs=[],
                            sync_info=mybir.SyncInfo(
                                on_wait=extra[g : g + 2], on_update=[]
                            ),
                        )
                        nc.register_instruction(ev)
                        il.insert(idx, ev)
                        idx += 1
                idx += 1
    return n_ev


class _G:
    __slots__ = ("kv", "qt", "outsb", "wrep", "abct", "pac", "b", "gi")


def _build(nb=NB, hg=HG, n_iters=N_ITERS):
    import concourse.bass as bass
    import concourse.mybir as mybir
    from concourse.tile import TileContext

    f32 = mybir.dt.float32
    bf16 = mybir.dt.bfloat16
    fp8 = mybir.dt.float8e4
    Copy = mybir.ActivationFunctionType.Copy
    mult = mybir.AluOpType.mult
    add = mybir.AluOpType.add
    DR = mybir.MatmulPerfMode.DoubleRow

    ngroups = nb * hg  # 12
    nwaves = ngroups // WAVE  # 3
    slots_per_wave = WAVE * n_iters  # 64
    n_tb = n_iters // CB  # 4 t-blocks per group

    nc = bass.Bass()
    q_d = nc.declare_dram_parameter(
        "qt", [nb, hg, 128, n_iters * 256], bf16, isOutput=False
    )
    kv_d = nc.declare_dram_parameter(
        "kv", [nb, hg, 128, n_iters * 2 * 4 * D], fp8, isOutput=False
    )
    w_d = nc.declare_dram_parameter("W12bd", [128, hg * 128], f32, isOutput=False)
    id_d = nc.declare_dram_parameter("ident", [128, 128], bf16, isOutput=False)
    out_d = nc.declare_dram_parameter(
        "out", [nb, hg, 128, n_iters * 256], bf16, isOutput=True
    )

    with TileContext(nc) as tc:
        with (
            tc.tile_pool(name="singles", bufs=1) as singles,
            tc.tile_pool(name="kv", bufs=8) as kv_pool,
            tc.tile_pool(name="qt", bufs=7) as qt_pool,
            tc.tile_pool(name="osb", bufs=5) as osb_pool,
            tc.tile_pool(name="abct", bufs=8) as abct_pool,
            tc.tile_pool(name="wrp", bufs=14) as wrp_pool,
            tc.tile_pool(name="pac", bufs=2, space="PSUM") as pac_pool,
            tc.tile_pool(name="pg", bufs=2, space="PSUM") as pg_pool,
            tc.tile_pool(name="pout", bufs=2, space="PSUM") as pout_pool,
        ):
            winit = singles.tile([128, hg, 128], f32)
            nc.sync.dma_start(
                out=winit, in_=w_d.rearrange("p (g e) -> p g e", g=hg)
            )
            ident = singles.tile([128, 128], bf16)
            nc.sync.dma_start(out=ident, in_=id_d[:, :])

            # persistent abct rotation: casts only ever write the diag
            # blocks, so the one-time memset zeros persist across reuses
            # (same logical tensors, manual rotation).
            abct_tiles = []
            for _ in range(8):
                t = abct_pool.tile([128, CB, 2, 128], bf16, tag="abct")
                nc.gpsimd.memset(t, 0.0)
                abct_tiles.append(t)
            abct_ctr = [0]

            glist = [None] * ngroups

            def ensure_group(gidx):
                if glist[gidx] is not None:
                    return
                g = _G()
                g.b, g.gi = divmod(gidx, hg)
                g.wrep = wrp_pool.tile([128, 128], bf16, tag="wrep")
                nc.vector.tensor_copy(g.wrep, winit[:, g.gi, :])
                g.abct = {}
                g.pac = None
                g.kv = None
                g.qt = None
                g.outsb = None
                glist[gidx] = g

            def _q(gidx):
                # alternate DMA trigger queues so DGE generation overlaps
                return nc.sync if gidx % 2 == 0 else nc.gpsimd

            def kv_part(gidx, h, nh):
                # kv DMA in 1/nh fractions (latency vs per-transfer overhead)
                ensure_group(gidx)
                g = glist[gidx]
                if g.kv is None:
                    g.kv = kv_pool.tile(
                        [128, n_iters, 2, 4, D], fp8, tag="kv"
                    )
                hc = n_iters // nh
                w2 = hc * 2 * 4 * D
                _q(gidx).dma_start(
                    out=g.kv[:, h * hc : (h + 1) * hc, :, :, :],
                    in_=kv_d[g.b, g.gi, :, h * w2 : (h + 1) * w2].rearrange(
                        "p (c j s d) -> p c j s d", j=2, s=4, d=D
                    ),
                )

            def qt_part(gidx, h, nh):
                g = glist[gidx]
                if g.qt is None:
                    g.qt = qt_pool.tile([128, n_iters, 256], bf16, tag="qt")
                hc = n_iters // nh
                w2 = hc * 256
                _q(gidx).dma_start(
                    out=g.qt[:, h * hc : (h + 1) * hc, :],
                    in_=q_d[g.b, g.gi, :, h * w2 : (h + 1) * w2].rearrange(
                        "p (c t) -> p c t", t=256
                    ),
                )

            def emit_act(gidx, tb, u):
                # chunk c = CB*tb + u Gram matmul into pac[:, u, :, :]
                g = glist[gidx]
                c = CB * tb + u
                if u == 0:
                    g.pac = pac_pool.tile([128, CB, 2, 128], f32, tag="pac")
                if USE_DR:
                    nc.tensor.matmul(
                        g.pac[:, u, :, :],
                        lhsT=g.kv[:, c, :, 0:2, :],
                        rhs=g.kv[:, c, :, :, :],
                        start=True, stop=True,
                        perf_mode=DR,
                        skip_group_check=True,
                    )
                else:
                    for j in (0, 1):
                        nc.tensor.matmul(
                            g.pac[:, u, :, :],
                            lhsT=g.kv[:, c, j, 0:2, :],
                            rhs=g.kv[:, c, j, :, :],
                            start=(j == 0), stop=(j == 1),
                            skip_group_check=True,
                        )

            def emit_cast(gidx, tb):
                # A/ct diag blocks -> block-diag bf16 (abct off-diag stays 0)
                g = glist[gidx]
                ab = abct_tiles[abct_ctr[0] % len(abct_tiles)]
                on_dve = abct_ctr[0] % 4 == 3
                abct_ctr[0] += 1
                nc.scalar.activation(
                    ab[0:64, :, :, 0:64], g.pac[0:64, :, :, 0:64],
                    func=Copy, scale=1.0,
                )
                if on_dve:
                    nc.vector.tensor_copy(
                        ab[64:128, :, :, 64:128], g.pac[64:128, :, :, 64:128]
                    )
                else:
                    nc.scalar.activation(
                        ab[64:128, :, :, 64:128], g.pac[64:128, :, :, 64:128],
                        func=Copy, scale=1.0,
                    )
                g.abct[tb] = ab
                g.pac = None

            def chain_seed(g, c):
                tb, u = divmod(c, CB)
                ab = g.abct[tb]
                pg = pg_pool.tile([128, 512], f32, tag="pg")
                nc.tensor.matmul(
                    pg[:, 0:128],
                    lhsT=ident[:, :],
                    rhs=ab[:, u, 1, :],
                    start=True, stop=False, skip_group_check=True,
                )
                return pg

            def chain_grad(g, c, pg):
                tb, u = divmod(c, CB)
                ab = g.abct[tb]
                nc.tensor.matmul(
                    pg[:, 0:128],
                    lhsT=ab[:, u, 0, :],
                    rhs=g.wrep[:, :],
                    start=False, stop=True, skip_group_check=True,
                )
                wnew = wrp_pool.tile([128, 128], bf16, tag="wrep")
                nc.vector.scalar_tensor_tensor(
                    wnew, pg[:, 0:128], -SCALE, g.wrep,
                    op0=mult, op1=add,
                )
                g.wrep = wnew
                if u == CB - 1:
                    del g.abct[tb]

            def emit_out_mm(gidx, c, wrep, po, slot_idx):
                g = glist[gidx]
                nc.tensor.matmul(
                    po[:, slot_idx, :], lhsT=wrep[:, :], rhs=g.qt[:, c, :],
                    start=True, stop=True, skip_group_check=True,
                )

            def emit_evac(gidx, c0, po, evac_on_act):
                g = glist[gidx]
                if g.outsb is None:
                    g.outsb = osb_pool.tile(
                        [128, n_iters, 256], bf16, tag="osb"
                    )
                dst = g.outsb[:, c0 : c0 + 2, :]
                if evac_on_act:
                    nc.scalar.activation(dst, po, func=Copy, scale=1.0)
                else:
                    nc.vector.tensor_copy(dst, po)
                half = n_iters * 256 // 2
                oq = nc.gpsimd if gidx % 2 == 0 else nc.sync
                if c0 + 1 == n_iters // 2 - 1:
                    oq.dma_start(
                        out=out_d[g.b, g.gi, :, 0:half],
                        in_=g.outsb[:, 0 : n_iters // 2, :],
                    )
                elif c0 + 1 == n_iters - 1:
                    oq.dma_start(
                        out=out_d[g.b, g.gi, :, half : 2 * half],
                        in_=g.outsb[:, n_iters // 2 : n_iters, :],
                    )
                    g.outsb = None
                    g.qt = None
                    g.kv = None

            # ---------------- schedule -----------------------------------
            # chain slot s (0..191): wave w = s//64, r = s%64, c = r//WAVE,
            #   gp = r%WAVE, group g = w*WAVE+gp.
            # act item (g, tb): 4 DR matmuls at slots w*64+16*tb+gp-LAG ...
            #   +3, cast at +4.
            # group kv DMA one wave ahead (spread), qt half a wave ahead.
            events = {}

            def at(slot, fn, *args):
                events.setdefault(slot, []).append((fn, args))

            n_slots = nwaves * slots_per_wave
            for w in range(nwaves):
                for gp in range(WAVE):
                    gidx = w * WAVE + gp
                    if w == 0:
                        # wave 0: quarter-kv first (acts gate on the least
                        # bytes), alternating trigger queues by gp parity
                        at(-40 + gp, kv_part, gidx, 0, 4)
                        at(-36 + gp, kv_part, gidx, 1, 4)
                        at(-32 + 2 * gp, qt_part, gidx, 0, 2)
                        at(-24 + 2 * gp, kv_part, gidx, 2, 4)
                        at(-16 + 2 * gp, kv_part, gidx, 3, 4)
                        at(-8 + 2 * gp, qt_part, gidx, 1, 2)
                    else:
                        at(w * 64 - 76 + 8 * gp, kv_part, gidx, 0, 1)
                        at(w * 64 - 72 + 8 * gp, qt_part, gidx, 0, 1)
                    for tb in range(n_tb):
                        t0 = w * 64 + 16 * tb + 4 * gp - LAG
                        for u in range(CB):
                            at(t0 + u, emit_act, gidx, tb, u)
                        at(t0 + CB, emit_cast, gidx, tb)

            # pending out-pairs: (gidx, c0, w0, w1)
            pend = []
            prev_w = [None] * ngroups
            evac_flip = [0]

            lo = min(events)
            for s in range(lo, n_slots + 3):
                for fn, args in events.get(s, ()):
                    fn(*args)
                # interleave within the slot so no two consecutive matmuls
                # target the same PSUM bank, and DVE sees stt BEFORE evac:
                #   out(c0) [pout] .. seed [pg] .. out(c0+1) [pout]
                #   .. grad [pg] + stt .. evac
                po_info = None
                if pend and s >= 6:  # wave-0 warmup: let qt DMA land first
                    gq, c0q, w0q, w1q = pend.pop(0)
                    evac_flip[0] = (evac_flip[0] + 1) % 3
                    on_act = evac_flip[0] != 0  # 2/3 Act, 1/3 DVE
                    po = pout_pool.tile([128, 2, 256], f32, tag="po")
                    emit_out_mm(gq, c0q, w0q, po, 0)
                    po_info = (gq, c0q, w1q, po, on_act)
                in_chain = 0 <= s < n_slots
                if in_chain:
                    w, r = divmod(s, slots_per_wave)
                    c, gp = divmod(r, WAVE)
                    gidx = w * WAVE + gp
                    g = glist[gidx]
                    pg = chain_seed(g, c)
                if po_info is not None:
                    emit_out_mm(po_info[0], po_info[1] + 1, po_info[2],
                                po_info[3], 1)
                if in_chain:
                    chain_grad(g, c, pg)
                    if c % 2 == 1:
                        pend.append((gidx, c - 1, prev_w[gidx], g.wrep))
                    else:
                        prev_w[gidx] = g.wrep
                if po_info is not None:
                    emit_evac(po_info[0], po_info[1], po_info[3], po_info[4])
            while pend:
                gq, c0q, w0q, w1q = pend.pop(0)
                evac_flip[0] = (evac_flip[0] + 1) % 3
                po = pout_pool.tile([128, 2, 256], f32, tag="po")
                emit_out_mm(gq, c0q, w0q, po, 0)
                emit_out_mm(gq, c0q + 1, w1q, po, 1)
                emit_evac(gq, c0q, po, evac_flip[0] != 0)

    _split_excess_waits(nc)
    return nc


def _get_nc():
    if "nc" not in _CACHE:
        _CACHE["nc"] = _build()
    return _CACHE["nc"]


def _host_prep(q, k, v):
    """Host re-layout (token t = c*256 + j*128 + p)."""
    import ml_dtypes

    bf = ml_dtypes.bfloat16
    f8 = ml_dtypes.float8_e4m3
    Bq, Hq, Nq, Dq = q.shape
    hg = Hq // 2
    ni = Nq // 256
    # kv: [b, g, p, c, j, (k0|k1|v0|v1), d]
    k7 = k.reshape(Bq, hg, 2, ni, 2, 128, Dq)
    v7 = (-v).reshape(Bq, hg, 2, ni, 2, 128, Dq)
    kv = np.stack(
        [k7[:, :, 0], k7[:, :, 1], v7[:, :, 0], v7[:, :, 1]], axis=5
    )  # [b, g, c, j, p, 4, d]
    kv = np.ascontiguousarray(
        kv.transpose(0, 1, 4, 2, 3, 5, 6).reshape(Bq, hg, 128, ni * 2 * 4 * Dq)
    ).astype(f8)
    # qt: [b, g, hpair*64+d, c, t]
    q6 = q.reshape(Bq, hg, 2, ni, 256, Dq)
    qt = np.ascontiguousarray(
        q6.transpose(0, 1, 2, 5, 3, 4).reshape(Bq, hg, 128, ni * 256)
    ).astype(bf)
    return kv, qt


def _host_unshuffle(out_host):
    """[b, g, hpair*64+e, c*256+t] bf16 -> (B, N, H*64) f32."""
    Bq, hgq, _, w = out_host.shape
    ni = w // 256
    o6 = np.asarray(out_host, dtype=np.float32).reshape(
        Bq, hgq, 2, 64, ni, 256
    )
    # [b, g, hp, e, c, t] -> [b, c, t, g, hp, e]
    return np.ascontiguousarray(
        o6.transpose(0, 4, 5, 1, 2, 3).reshape(Bq, ni * 256, hgq * 2 * 64)
    )


def kernel(q, k, v, W_init, training=0, return_aux=0, **_unused):
    import ml_dtypes
    from concourse.bass_utils import run_bass_kernel_spmd

    q = np.asarray(q, dtype=np.float32)
    k = np.asarray(k, dtype=np.float32)
    v = np.asarray(v, dtype=np.float32)
    W_init = np.ascontiguousarray(np.asarray(W_init, dtype=np.float32))

    kv, qt = _host_prep(q, k, v)
    wbd = np.zeros((HG, 128, 128), dtype=np.float32)
    wbd[:, 0:64, 0:64] = W_init[0::2]
    wbd[:, 64:128, 64:128] = W_init[1::2]
    wbd = np.ascontiguousarray(
        wbd.transpose(1, 0, 2).reshape(128, HG * 128)
    )
    ident = np.eye(128, dtype=ml_dtypes.bfloat16)

    nc = _get_nc()
    in_maps = []
    for i in range(NCORES):
        sl = slice(i * NB, (i + 1) * NB)
        in_maps.append(
            {"qt": qt[sl], "kv": kv[sl], "W12bd": wbd, "ident": ident}
        )

    trace = bool(int(os.environ.get("BASS_KERNEL_TRACE", "0")))
    res = run_bass_kernel_spmd(
        nc, in_maps, core_ids=list(range(NCORES)), trace=trace
    )
    _CACHE["last_results"] = res
    out_host = np.concatenate(
        [np.asarray(res.results[i]["out"]) for i in range(NCORES)], axis=0
    )
    return _host_unshuffle(out_host)


if __name__ == "__main__":
    rng = np.random.default_rng(0)
    q = rng.standard_normal((B, H, N, D), dtype=np.float32)
    k = rng.standard_normal((B, H, N, D), dtype=np.float32)
    v = rng.standard_normal((B, H, N, D), dtype=np.float32)
    W = (rng.standard_normal((H, D, D)) * D**-0.5).astype(np.float32)
    out = kernel(q, k, v, W)
    print("kernel ran, out shape:", out.shape)


# revision 29
# speedup vs baseline: 1.0391x; 1.0391x over previous
"""Trainium2 Bass kernel for ARM TTT multi-head self-attention (inner-GD scan).

Math per (b, h) pair (B=16, H=12, N=4096, D=64, 16 chunks of m=256 tokens):
    A_i = k_i^T k_i ;  ct_i = k_i^T (-v_i)      (token contraction)
    grad_raw_i = A_i @ W_{i-1} + ct_i
    W_i = W_{i-1} - s * grad_raw_i,  s = 1/(m*D)
    out_i = q_i @ W_i
Pairs are fully independent -> shard B over the 8 NeuronCores (24 chains/core).

v6: v5's measured bottleneck was PE instruction CADENCE (~116ns per matmul
regardless of size: LdWeights + dispatch), 12 matmuls per chunk.  v6 packs
each head-PAIR into block-diagonal 128x128 operands -> 4 matmuls per chunk:

  1. act:  ONE fp8 DoubleRow matmul per chunk contracts all 256 tokens:
           lhsT = [k0|k1] (128t x 2j x 128), rhs = [k0|k1|v0|v1] (x 256)
           -> pac[128, 256]: A0/A1 diag blocks of cols 0:128, ct0/ct1 diag
              blocks of cols 128:256 (junk off-diag).  4 chunks per pac.
  2. cast: per t-block (4 chunks), per pair, ONE activation moves the A/ct
           diag blocks into PERSISTENT pre-zeroed block-diag bf16 tiles
           (abct) - zeros off the diag keep the chain closed in block-diag.
  3. seed: matmul(pg = Id^T @ ctbd)   [start of PSUM accumulation group]
  4. grad: matmul(pg += Abd^T @ Wbd)  [stop]
     stt (DVE, ONE op): Wbd' = -s*pg + Wbd   (off-diag stays 0: 0*s+0)
  5. out:  matmul(pout = Wbd'(lhsT) @ qt[128 dpair x 256 tok]) - both heads
           in one 256-col matmul; emitted in chunk-PAIRS, one evac per pair.

The serial W-chain round trip (PE->DVE->PE ~1us) is hidden by round-robining
chunks across a 4-group window; acts/casts lead by LAG slots; outs trail.
PSUM budget (8 banks): pac 2x2 + pg 2 + pout 2.
GpSimd CANNOT access PSUM on TRN2, so casts go to Act and evacs to DVE/Act.

Device layouts (token t = c*256 + j*128 + p):
    kv (per group):  [128(p), 16(c), 2(j), 4(k0|k1|v0|v1), 64]  fp8 (v negated)
    qt (per group):  [128(hpair*64+d), 16(c), 256(t=j*128+p)]   bf16
    out (per group): [128(hpair*64+e), 16(c), 256(t)]           bf16
    W12bd: [hg, 128, 128] f32 block-diag(W_h0, W_h1); carried chain is bf16.
"""

import os
import sys

sys.path.insert(0, "/opt/trn_rl_repo")

import numpy as np

B, H, N, D = 16, 12, 4096, 64
N_ITERS = 16
M = N // N_ITERS  # 256 tokens per chunk
NCORES = 8
NB = B // NCORES  # batches per core
HG = H // 2  # head-pair groups per batch
SCALE = 1.0 / (M * D)
WAVE = 4  # chain interleave width (groups round-robined per chunk)
LAG = 14  # slots the act/cast stream leads the chain stream
CB = 4  # chunks per t-block (pac granularity)
USE_DR = True  # fp8 DoubleRow: one act matmul per chunk (else 2, j-accum)

_CACHE = {}


def _split_excess_waits(nc):
    """walrus in this env accepts at most ONE sem wait per instruction
    (two on EventSemaphore); move excess waits onto EventSemaphore
    instructions inserted just before on the same engine."""
    import concourse.mybir as mybir

    n_ev = 0
    for f in nc.m.functions:
        for b in f.blocks:
            il = b.instructions
            idx = 0
            while idx < len(il):
                inst = il[idx]
                si = getattr(inst, "sync_info", None)
                if si is not None and len(si.on_wait) > 1:
                    waits = list(si.on_wait)
                    si.on_wait = [waits[0]]
                    extra = waits[1:]
                    for g in range(0, len(extra), 2):
                        n_ev += 1
                        ev = mybir.InstEventSemaphore(
                            name=f"EVSPLIT-{n_ev}",
                            engine=inst.engine,
                            ins=[],
                            outs=[],
                            sync_info=mybir.SyncInfo(
                                on_wait=extra[g : g + 2], on_update=[]
                            ),
                        )
                        nc.register_instruction(ev)
                        il.insert(idx, ev)
                        idx += 1
                idx += 1
    return n_ev


class _G:
    __slots__ = ("kv", "qt", "outsb", "wrep", "abct", "pac", "b", "gi")


def _build(nb=NB, hg=HG, n_iters=N_ITERS):
    import concourse.bass as bass
    import concourse.mybir as mybir
    from concourse.tile import TileContext

    f32 = mybir.dt.float32
    bf16 = mybir.dt.bfloat16
    fp8 = mybir.dt.float8e4
    Copy = mybir.ActivationFunctionType.Copy
    mult = mybir.AluOpType.mult
    add = mybir.AluOpType.add
    DR = mybir.MatmulPerfMode.DoubleRow

    ngroups = nb * hg  # 12
    nwaves = ngroups // WAVE  # 3
    slots_per_wave = WAVE * n_iters  # 64
    n_tb = n_iters // CB  # 4 t-blocks per group

    nc = bass.Bass()
    q_d = nc.declare_dram_parameter(
        "qt", [nb, hg, 128, n_iters * 256], bf16, isOutput=False
    )
    kv_d = nc.declare_dram_parameter(
        "kv", [nb, hg, 128, n_iters * 2 * 4 * D], fp8, isOutput=False
    )
    w_d = nc.declare_dram_parameter("W12bd", [128, hg * 128], f32, isOutput=False)
    id_d = nc.declare_dram_parameter("ident", [128, 128], bf16, isOutput=False)
    out_d = nc.declare_dram_parameter(
        "out", [nb, hg, 128, n_iters * 256], bf16, isOutput=True
    )

    with TileContext(nc) as tc:
        with (
            tc.tile_pool(name="singles", bufs=1) as singles,
            tc.tile_pool(name="kv", bufs=8) as kv_pool,
            tc.tile_pool(name="qt", bufs=7) as qt_pool,
            tc.tile_pool(name="osb", bufs=5) as osb_pool,
            tc.tile_pool(name="abct", bufs=8) as abct_pool,
            tc.tile_pool(name="wrp", bufs=14) as wrp_pool,
            tc.tile_pool(name="pac", bufs=2, space="PSUM") as pac_pool,
            tc.tile_pool(name="pg", bufs=2, space="PSUM") as pg_pool,
            tc.tile_pool(name="pout", bufs=2, space="PSUM") as pout_pool,
        ):
            winit = singles.tile([128, hg, 128], f32)
            nc.sync.dma_start(
                out=winit, in_=w_d.rearrange("p (g e) -> p g e", g=hg)
            )
            ident = singles.tile([128, 128], bf16)
            nc.sync.dma_start(out=ident, in_=id_d[:, :])

            # persistent abct rotation: casts only ever write the diag
            # blocks, so the one-time memset zeros persist across reuses
            # (same logical tensors, manual rotation).  The memsets are
            # emitted from the schedule (after wave-0 DMA triggers) so they
            # don't block the GpSimd DMA queue at startup.
            abct_tiles = []
            for _i in range(8):
                abt = abct_pool.tile([128, CB, 2, 128], bf16, tag="abct")
                abct_tiles.append(abt)
            abct_ctr = [0]

            def memset_abct(i):
                nc.gpsimd.memset(abct_tiles[i], 0.0)

            glist = [None] * ngroups

            def ensure_group(gidx):
                if glist[gidx] is not None:
                    return
                g = _G()
                g.b, g.gi = divmod(gidx, hg)
                g.wrep = wrp_pool.tile([128, 128], bf16, tag="wrep")
                nc.vector.tensor_copy(g.wrep, winit[:, g.gi, :])
                g.abct = {}
                g.pac = None
                g.kv = None
                g.qt = None
                g.outsb = None
                glist[gidx] = g

            def _q(gidx):
                # alternate DMA trigger queues so DGE generation overlaps
                return nc.sync if gidx % 2 == 0 else nc.gpsimd

            def kv_part(gidx, h, nh):
                # kv DMA in 1/nh fractions (latency vs per-transfer overhead)
                ensure_group(gidx)
                g = glist[gidx]
                if g.kv is None:
                    g.kv = kv_pool.tile(
                        [128, n_iters, 2, 4, D], fp8, tag="kv"
                    )
                hc = n_iters // nh
                w2 = hc * 2 * 4 * D
                _q(gidx).dma_start(
                    out=g.kv[:, h * hc : (h + 1) * hc, :, :, :],
                    in_=kv_d[g.b, g.gi, :, h * w2 : (h + 1) * w2].rearrange(
                        "p (c j s d) -> p c j s d", j=2, s=4, d=D
                    ),
                )

            def qt_part(gidx, h, nh):
                g = glist[gidx]
                if g.qt is None:
                    g.qt = qt_pool.tile([128, n_iters, 256], bf16, tag="qt")
                hc = n_iters // nh
                w2 = hc * 256
                _q(gidx).dma_start(
                    out=g.qt[:, h * hc : (h + 1) * hc, :],
                    in_=q_d[g.b, g.gi, :, h * w2 : (h + 1) * w2].rearrange(
                        "p (c t) -> p c t", t=256
                    ),
                )

            def emit_act(gidx, tb, u):
                # chunk c = CB*tb + u Gram matmul into pac[:, u, :, :]
                g = glist[gidx]
                c = CB * tb + u
                if u == 0:
                    g.pac = pac_pool.tile([128, CB, 2, 128], f32, tag="pac")
                if USE_DR:
                    nc.tensor.matmul(
                        g.pac[:, u, :, :],
                        lhsT=g.kv[:, c, :, 0:2, :],
                        rhs=g.kv[:, c, :, :, :],
                        start=True, stop=True,
                        perf_mode=DR,
                        skip_group_check=True,
                    )
                else:
                    for j in (0, 1):
                        nc.tensor.matmul(
                            g.pac[:, u, :, :],
                            lhsT=g.kv[:, c, j, 0:2, :],
                            rhs=g.kv[:, c, j, :, :],
                            start=(j == 0), stop=(j == 1),
                            skip_group_check=True,
                        )

            def emit_cast(gidx, tb):
                # A/ct diag blocks -> block-diag bf16 (abct off-diag stays 0)
                g = glist[gidx]
                ab = abct_tiles[abct_ctr[0] % len(abct_tiles)]
                on_dve = abct_ctr[0] % 4 == 3
                abct_ctr[0] += 1
                nc.scalar.activation(
                    ab[0:64, :, :, 0:64], g.pac[0:64, :, :, 0:64],
                    func=Copy, scale=1.0,
                )
                if on_dve:
                    nc.vector.tensor_copy(
                        ab[64:128, :, :, 64:128], g.pac[64:128, :, :, 64:128]
                    )
                else:
                    nc.scalar.activation(
                        ab[64:128, :, :, 64:128], g.pac[64:128, :, :, 64:128],
                        func=Copy, scale=1.0,
                    )
                g.abct[tb] = ab
                g.pac = None

            def chain_seed(g, c):
                tb, u = divmod(c, CB)
                ab = g.abct[tb]
                pg = pg_pool.tile([128, 512], f32, tag="pg")
                nc.tensor.matmul(
                    pg[:, 0:128],
                    lhsT=ident[:, :],
                    rhs=ab[:, u, 1, :],
                    start=True, stop=False, skip_group_check=True,
                )
                return pg

            def chain_grad(g, c, pg):
                tb, u = divmod(c, CB)
                ab = g.abct[tb]
                nc.tensor.matmul(
                    pg[:, 0:128],
                    lhsT=ab[:, u, 0, :],
                    rhs=g.wrep[:, :],
                    start=False, stop=True, skip_group_check=True,
                )
                wnew = wrp_pool.tile([128, 128], bf16, tag="wrep")
                nc.vector.scalar_tensor_tensor(
                    wnew, pg[:, 0:128], -SCALE, g.wrep,
                    op0=mult, op1=add,
                )
                g.wrep = wnew
                if u == CB - 1:
                    del g.abct[tb]

            def emit_out_mm(gidx, c, wrep, po, slot_idx):
                g = glist[gidx]
                nc.tensor.matmul(
                    po[:, slot_idx, :], lhsT=wrep[:, :], rhs=g.qt[:, c, :],
                    start=True, stop=True, skip_group_check=True,
                )

            def emit_evac(gidx, c0, po, evac_on_act):
                g = glist[gidx]
                if g.outsb is None:
                    g.outsb = osb_pool.tile(
                        [128, n_iters, 256], bf16, tag="osb"
                    )
                dst = g.outsb[:, c0 : c0 + 2, :]
                if evac_on_act:
                    nc.scalar.activation(dst, po, func=Copy, scale=1.0)
                else:
                    nc.vector.tensor_copy(dst, po)
                half = n_iters * 256 // 2
                oq = nc.gpsimd if gidx % 2 == 0 else nc.sync
                if c0 + 1 == n_iters // 2 - 1:
                    oq.dma_start(
                        out=out_d[g.b, g.gi, :, 0:half],
                        in_=g.outsb[:, 0 : n_iters // 2, :],
                    )
                elif c0 + 1 == n_iters - 1:
                    oq.dma_start(
                        out=out_d[g.b, g.gi, :, half : 2 * half],
                        in_=g.outsb[:, n_iters // 2 : n_iters, :],
                    )
                    g.outsb = None
                    g.qt = None
                    g.kv = None

            # ---------------- schedule -----------------------------------
            # chain slot s (0..191): wave w = s//64, r = s%64, c = r//WAVE,
            #   gp = r%WAVE, group g = w*WAVE+gp.
            # act item (g, tb): 4 DR matmuls at slots w*64+16*tb+gp-LAG ...
            #   +3, cast at +4.
            # group kv DMA one wave ahead (spread), qt half a wave ahead.
            events = {}

            def at(slot, fn, *args):
                events.setdefault(slot, []).append((fn, args))

            n_slots = nwaves * slots_per_wave
            for w in range(nwaves):
                for gp in range(WAVE):
                    gidx = w * WAVE + gp
                    if w == 0:
                        # wave 0: kv halves first (acts gate on the least
                        # bytes), alternating trigger queues by gp parity
                        at(-40 + gp, kv_part, gidx, 0, 2)
                        at(-32 + 2 * gp, qt_part, gidx, 0, 2)
                        at(-24 + 2 * gp, kv_part, gidx, 1, 2)
                        at(-8 + 2 * gp, qt_part, gidx, 1, 2)
                    else:
                        at(w * 64 - 76 + 8 * gp, kv_part, gidx, 0, 1)
                        at(w * 64 - 72 + 8 * gp, qt_part, gidx, 0, 1)
                    for tb in range(n_tb):
                        t0 = w * 64 + 16 * tb + 4 * gp - LAG
                        for u in range(CB):
                            at(t0 + u, emit_act, gidx, tb, u)
                        at(t0 + CB, emit_cast, gidx, tb)
            for i, sl in enumerate((-36, -35, -25, -24, -17, -16, -12, -11)):
                at(sl, memset_abct, i)

            # pending out-pairs: (gidx, c0, w0, w1)
            pend = []
            prev_w = [None] * ngroups
            evac_flip = [0]

            lo = min(events)
            for s in range(lo, n_slots + 3):
                for fn, args in events.get(s, ()):
                    fn(*args)
                # interleave within the slot so no two consecutive matmuls
                # target the same PSUM bank, and DVE sees stt BEFORE evac:
                #   out(c0) [pout] .. seed [pg] .. out(c0+1) [pout]
                #   .. grad [pg] + stt .. evac
                po_info = None
                if pend and s >= 6:  # wave-0 warmup: let qt DMA land first
                    gq, c0q, w0q, w1q = pend.pop(0)
                    evac_flip[0] = (evac_flip[0] + 1) % 3
                    on_act = evac_flip[0] != 0  # 2/3 Act, 1/3 DVE
                    po = pout_pool.tile([128, 2, 256], f32, tag="po")
                    emit_out_mm(gq, c0q, w0q, po, 0)
                    po_info = (gq, c0q, w1q, po, on_act)
                in_chain = 0 <= s < n_slots
                if in_chain:
                    w, r = divmod(s, slots_per_wave)
                    c, gp = divmod(r, WAVE)
                    gidx = w * WAVE + gp
                    g = glist[gidx]
                    pg = chain_seed(g, c)
                if po_info is not None:
                    emit_out_mm(po_info[0], po_info[1] + 1, po_info[2],
                                po_info[3], 1)
                if in_chain:
                    chain_grad(g, c, pg)
                    if c % 2 == 1:
                        pend.append((gidx, c - 1, prev_w[gidx], g.wrep))
                    else:
                        prev_w[gidx] = g.wrep
                if po_info is not None:
                    emit_evac(po_info[0], po_info[1], po_info[3], po_info[4])
            while pend:
                gq, c0q, w0q, w1q = pend.pop(0)
                evac_flip[0] = (evac_flip[0] + 1) % 3
                po = pout_pool.tile([128, 2, 256], f32, tag="po")
                emit_out_mm(gq, c0q, w0q, po, 0)
                emit_out_mm(gq, c0q + 1, w1q, po, 1)
                emit_evac(gq, c0q, po, evac_flip[0] != 0)

    _split_excess_waits(nc)
    return nc


def _get_nc():
    if "nc" not in _CACHE:
        _CACHE["nc"] = _build()
    return _CACHE["nc"]


def _host_prep(q, k, v):
    """Host re-layout (token t = c*256 + j*128 + p)."""
    import ml_dtypes

    bf = ml_dtypes.bfloat16
    f8 = ml_dtypes.float8_e4m3
    Bq, Hq, Nq, Dq = q.shape
    hg = Hq // 2
    ni = Nq // 256
    # kv: [b, g, p, c, j, (k0|k1|v0|v1), d]
    k7 = k.reshape(Bq, hg, 2, ni, 2, 128, Dq)
    v7 = (-v).reshape(Bq, hg, 2, ni, 2, 128, Dq)
    kv = np.stack(
        [k7[:, :, 0], k7[:, :, 1], v7[:, :, 0], v7[:, :, 1]], axis=5
    )  # [b, g, c, j, p, 4, d]
    kv = np.ascontiguousarray(
        kv.transpose(0, 1, 4, 2, 3, 5, 6).reshape(Bq, hg, 128, ni * 2 * 4 * Dq)
    ).astype(f8)
    # qt: [b, g, hpair*64+d, c, t]
    q6 = q.reshape(Bq, hg, 2, ni, 256, Dq)
    qt = np.ascontiguousarray(
        q6.transpose(0, 1, 2, 5, 3, 4).reshape(Bq, hg, 128, ni * 256)
    ).astype(bf)
    return kv, qt


def _host_unshuffle(out_host):
    """[b, g, hpair*64+e, c*256+t] bf16 -> (B, N, H*64) f32."""
    Bq, hgq, _, w = out_host.shape
    ni = w // 256
    o6 = np.asarray(out_host, dtype=np.float32).reshape(
        Bq, hgq, 2, 64, ni, 256
    )
    # [b, g, hp, e, c, t] -> [b, c, t, g, hp, e]
    return np.ascontiguousarray(
        o6.transpose(0, 4, 5, 1, 2, 3).reshape(Bq, ni * 256, hgq * 2 * 64)
    )


def kernel(q, k, v, W_init, training=0, return_aux=0, **_unused):
    import ml_dtypes
    from concourse.bass_utils import run_bass_kernel_spmd

    q = np.asarray(q, dtype=np.float32)
    k = np.asarray(k, dtype=np.float32)
    v = np.asarray(v, dtype=np.float32)
    W_init = np.ascontiguousarray(np.asarray(W_init, dtype=np.float32))

    kv, qt = _host_prep(q, k, v)
    wbd = np.zeros((HG, 128, 128), dtype=np.float32)
    wbd[:, 0:64, 0:64] = W_init[0::2]
    wbd[:, 64:128, 64:128] = W_init[1::2]
    wbd = np.ascontiguousarray(
        wbd.transpose(1, 0, 2).reshape(128, HG * 128)
    )
    ident = np.eye(128, dtype=ml_dtypes.bfloat16)

    nc = _get_nc()
    in_maps = []
    for i in range(NCORES):
        sl = slice(i * NB, (i + 1) * NB)
        in_maps.append(
            {"qt": qt[sl], "kv": kv[sl], "W12bd": wbd, "ident": ident}
        )

    trace = bool(int(os.environ.get("BASS_KERNEL_TRACE", "0")))
    res = run_bass_kernel_spmd(
        nc, in_maps, core_ids=list(range(NCORES)), trace=trace
    )
    _CACHE["last_results"] = res
    out_host = np.concatenate(
        [np.asarray(res.results[i]["out"]) for i in range(NCORES)], axis=0
    )
    return _host_unshuffle(out_host)


if __name__ == "__main__":
    rng = np.random.default_rng(0)
    q = rng.standard_normal((B, H, N, D), dtype=np.float32)
    k = rng.standard_normal((B, H, N, D), dtype=np.float32)
    v = rng.standard_normal((B, H, N, D), dtype=np.float32)
    W = (rng.standard_normal((H, D, D)) * D**-0.5).astype(np.float32)
    out = kernel(q, k, v, W)
    print("kernel ran, out shape:", out.shape)


# revision 33
# speedup vs baseline: 1.1105x; 1.0688x over previous
"""Trainium2 Bass kernel for ARM TTT multi-head self-attention (inner-GD scan).

Math per (b, h) pair (B=16, H=12, N=4096, D=64, 16 chunks of m=256 tokens):
    A_i = k_i^T k_i ;  ct_i = k_i^T (-v_i)      (token contraction)
    grad_raw_i = A_i @ W_{i-1} + ct_i
    W_i = W_{i-1} - s * grad_raw_i,  s = 1/(m*D)
    out_i = q_i @ W_i
Pairs are fully independent -> shard B over the 8 NeuronCores (24 chains/core).

v6: v5's measured bottleneck was PE instruction CADENCE (~116ns per matmul
regardless of size: LdWeights + dispatch), 12 matmuls per chunk.  v6 packs
each head-PAIR into block-diagonal 128x128 operands -> 4 matmuls per chunk:

  1. act:  ONE fp8 DoubleRow matmul per chunk contracts all 256 tokens:
           lhsT = [k0|k1] (128t x 2j x 128), rhs = [k0|k1|v0|v1] (x 256)
           -> pac[128, 256]: A0/A1 diag blocks of cols 0:128, ct0/ct1 diag
              blocks of cols 128:256 (junk off-diag).  4 chunks per pac.
  2. cast: per t-block (4 chunks), per pair, ONE activation moves the A/ct
           diag blocks into PERSISTENT pre-zeroed block-diag bf16 tiles
           (abct) - zeros off the diag keep the chain closed in block-diag.
  3. seed: matmul(pg = Id^T @ ctbd)   [start of PSUM accumulation group]
  4. grad: matmul(pg += Abd^T @ Wbd)  [stop]
     stt (DVE, ONE op): Wbd' = -s*pg + Wbd   (off-diag stays 0: 0*s+0)
  5. out:  matmul(pout = Wbd'(lhsT) @ qt[128 dpair x 256 tok]) - both heads
           in one 256-col matmul; emitted in chunk-PAIRS, one evac per pair.

The serial W-chain round trip (PE->DVE->PE ~1us) is hidden by round-robining
chunks across a 4-group window; acts/casts lead by LAG slots; outs trail.
PSUM budget (8 banks): pac 2x2 + pg 2 + pout 2.
GpSimd CANNOT access PSUM on TRN2, so casts go to Act and evacs to DVE/Act.

Device layouts (token t = c*256 + j*128 + p):
    kv (per group):  [128(p), 16(c), 2(j), 4(k0|k1|v0|v1), 64]  fp8 (v negated)
    qt (per group):  [128(hpair*64+d), 16(c), 256(t=j*128+p)]   bf16
    out (per group): [128(hpair*64+e), 16(c), 256(t)]           bf16
    W12bd: [hg, 128, 128] f32 block-diag(W_h0, W_h1); carried chain is bf16.
"""

import os
import sys

sys.path.insert(0, "/opt/trn_rl_repo")

import numpy as np

B, H, N, D = 16, 12, 4096, 64
N_ITERS = 16
M = N // N_ITERS  # 256 tokens per chunk
NCORES = 8
NB = B // NCORES  # batches per core
HG = H // 2  # head-pair groups per batch
SCALE = 1.0 / (M * D)
WAVE = 4  # chain interleave width (groups round-robined per chunk)
LAG = 14  # slots the act/cast stream leads the chain stream
CB = 4  # chunks per t-block (pac granularity)
USE_DR = True  # fp8 DoubleRow: one act matmul per chunk (else 2, j-accum)

_CACHE = {}


def _split_excess_waits(nc):
    """walrus in this env accepts at most ONE sem wait per instruction
    (two on EventSemaphore); move excess waits onto EventSemaphore
    instructions inserted just before on the same engine."""
    import concourse.mybir as mybir

    n_ev = 0
    for f in nc.m.functions:
        for b in f.blocks:
            il = b.instructions
            idx = 0
            while idx < len(il):
                inst = il[idx]
                si = getattr(inst, "sync_info", None)
                if si is not None and len(si.on_wait) > 1:
                    waits = list(si.on_wait)
                    si.on_wait = [waits[0]]
                    extra = waits[1:]
                    for g in range(0, len(extra), 2):
                        n_ev += 1
                        ev = mybir.InstEventSemaphore(
                            name=f"EVSPLIT-{n_ev}",
                            engine=inst.engine,
                            ins=[],
                            outs=[],
                            sync_info=mybir.SyncInfo(
                                on_wait=extra[g : g + 2], on_update=[]
                            ),
                        )
                        nc.register_instruction(ev)
                        il.insert(idx, ev)
                        idx += 1
                idx += 1
    return n_ev


class _G:
    __slots__ = ("kv", "qt", "outsb", "wrep", "abct", "pac", "b", "gi")


def _build(nb=NB, hg=HG, n_iters=N_ITERS):
    import concourse.bass as bass
    import concourse.mybir as mybir
    from concourse.tile import TileContext

    f32 = mybir.dt.float32
    bf16 = mybir.dt.bfloat16
    fp8 = mybir.dt.float8e4
    Copy = mybir.ActivationFunctionType.Copy
    mult = mybir.AluOpType.mult
    add = mybir.AluOpType.add
    DR = mybir.MatmulPerfMode.DoubleRow

    ngroups = nb * hg  # 12
    nwaves = ngroups // WAVE  # 3
    slots_per_wave = WAVE * n_iters  # 64
    n_tb = n_iters // CB  # 4 t-blocks per group

    nc = bass.Bass()
    q_d = nc.declare_dram_parameter(
        "qt", [nb, hg, 128, n_iters * 256], bf16, isOutput=False
    )
    kv_d = nc.declare_dram_parameter(
        "kv", [nb, hg, 128, n_iters * 2 * 4 * D], fp8, isOutput=False
    )
    w_d = nc.declare_dram_parameter("W12bd", [128, hg * 128], f32, isOutput=False)
    id_d = nc.declare_dram_parameter("ident", [128, 128], bf16, isOutput=False)
    out_d = nc.declare_dram_parameter(
        "out", [nb, hg, 128, n_iters * 256], bf16, isOutput=True
    )

    with TileContext(nc) as tc:
        with (
            tc.tile_pool(name="singles", bufs=1) as singles,
            tc.tile_pool(name="kv", bufs=8) as kv_pool,
            tc.tile_pool(name="qt", bufs=7) as qt_pool,
            tc.tile_pool(name="osb", bufs=5) as osb_pool,
            tc.tile_pool(name="abct", bufs=8) as abct_pool,
            tc.tile_pool(name="wrp", bufs=14) as wrp_pool,
            tc.tile_pool(name="pac", bufs=2, space="PSUM") as pac_pool,
            tc.tile_pool(name="pg", bufs=2, space="PSUM") as pg_pool,
            tc.tile_pool(name="pout", bufs=2, space="PSUM") as pout_pool,
        ):
            winit = singles.tile([128, hg, 128], f32)
            nc.sync.dma_start(
                out=winit, in_=w_d.rearrange("p (g e) -> p g e", g=hg)
            )
            ident = singles.tile([128, 128], bf16)
            nc.sync.dma_start(out=ident, in_=id_d[:, :])

            # persistent abct rotation: casts only ever write the diag
            # blocks, so the one-time memset zeros persist across reuses
            # (same logical tensors, manual rotation).  The memsets are
            # emitted from the schedule (after wave-0 DMA triggers) so they
            # don't block the GpSimd DMA queue at startup.
            abct_tiles = []
            for _i in range(8):
                abt = abct_pool.tile([128, CB, 2, 128], bf16, tag="abct")
                abct_tiles.append(abt)
            abct_ctr = [0]

            def memset_abct(i):
                nc.gpsimd.memset(abct_tiles[i], 0.0)

            glist = [None] * ngroups

            def ensure_group(gidx):
                if glist[gidx] is not None:
                    return
                g = _G()
                g.b, g.gi = divmod(gidx, hg)
                g.wrep = wrp_pool.tile([128, 128], bf16, tag="wrep")
                nc.vector.tensor_copy(g.wrep, winit[:, g.gi, :])
                g.abct = {}
                g.pac = None
                g.kv = None
                g.qt = None
                g.outsb = None
                glist[gidx] = g

            def _q(gidx):
                # single queue: transfers fair-share DMA engines, so issue
                # order IS the priority order - keep inputs on one queue
                return nc.sync

            def kv_part(gidx, h, nh):
                # kv DMA in 1/nh fractions (latency vs per-transfer overhead)
                ensure_group(gidx)
                g = glist[gidx]
                if g.kv is None:
                    g.kv = kv_pool.tile(
                        [128, n_iters, 2, 4, D], fp8, tag="kv"
                    )
                hc = n_iters // nh
                w2 = hc * 2 * 4 * D
                _q(gidx).dma_start(
                    out=g.kv[:, h * hc : (h + 1) * hc, :, :, :],
                    in_=kv_d[g.b, g.gi, :, h * w2 : (h + 1) * w2].rearrange(
                        "p (c j s d) -> p c j s d", j=2, s=4, d=D
                    ),
                )

            def qt_part(gidx, h, nh):
                g = glist[gidx]
                if g.qt is None:
                    g.qt = qt_pool.tile([128, n_iters, 256], bf16, tag="qt")
                hc = n_iters // nh
                w2 = hc * 256
                _q(gidx).dma_start(
                    out=g.qt[:, h * hc : (h + 1) * hc, :],
                    in_=q_d[g.b, g.gi, :, h * w2 : (h + 1) * w2].rearrange(
                        "p (c t) -> p c t", t=256
                    ),
                )

            def emit_act(gidx, tb, u):
                # chunk c = CB*tb + u Gram matmul into pac[:, u, :, :]
                g = glist[gidx]
                c = CB * tb + u
                if u == 0:
                    g.pac = pac_pool.tile([128, CB, 2, 128], f32, tag="pac")
                if USE_DR:
                    nc.tensor.matmul(
                        g.pac[:, u, :, :],
                        lhsT=g.kv[:, c, :, 0:2, :],
                        rhs=g.kv[:, c, :, :, :],
                        start=True, stop=True,
                        perf_mode=DR,
                        skip_group_check=True,
                    )
                else:
                    for j in (0, 1):
                        nc.tensor.matmul(
                            g.pac[:, u, :, :],
                            lhsT=g.kv[:, c, j, 0:2, :],
                            rhs=g.kv[:, c, j, :, :],
                            start=(j == 0), stop=(j == 1),
                            skip_group_check=True,
                        )

            def emit_cast(gidx, tb):
                # A/ct diag blocks -> block-diag bf16 (abct off-diag stays 0)
                g = glist[gidx]
                ab = abct_tiles[abct_ctr[0] % len(abct_tiles)]
                on_dve = abct_ctr[0] % 4 == 3
                abct_ctr[0] += 1
                nc.scalar.activation(
                    ab[0:64, :, :, 0:64], g.pac[0:64, :, :, 0:64],
                    func=Copy, scale=1.0,
                )
                if on_dve:
                    nc.vector.tensor_copy(
                        ab[64:128, :, :, 64:128], g.pac[64:128, :, :, 64:128]
                    )
                else:
                    nc.scalar.activation(
                        ab[64:128, :, :, 64:128], g.pac[64:128, :, :, 64:128],
                        func=Copy, scale=1.0,
                    )
                g.abct[tb] = ab
                g.pac = None

            def chain_seed(g, c):
                tb, u = divmod(c, CB)
                ab = g.abct[tb]
                pg = pg_pool.tile([128, 512], f32, tag="pg")
                nc.tensor.matmul(
                    pg[:, 0:128],
                    lhsT=ident[:, :],
                    rhs=ab[:, u, 1, :],
                    start=True, stop=False, skip_group_check=True,
                )
                return pg

            def chain_grad(g, c, pg):
                tb, u = divmod(c, CB)
                ab = g.abct[tb]
                nc.tensor.matmul(
                    pg[:, 0:128],
                    lhsT=ab[:, u, 0, :],
                    rhs=g.wrep[:, :],
                    start=False, stop=True, skip_group_check=True,
                )
                wnew = wrp_pool.tile([128, 128], bf16, tag="wrep")
                nc.vector.scalar_tensor_tensor(
                    wnew, pg[:, 0:128], -SCALE, g.wrep,
                    op0=mult, op1=add,
                )
                g.wrep = wnew
                if u == CB - 1:
                    del g.abct[tb]

            def emit_out_mm(gidx, c, wrep, po, slot_idx):
                g = glist[gidx]
                nc.tensor.matmul(
                    po[:, slot_idx, :], lhsT=wrep[:, :], rhs=g.qt[:, c, :],
                    start=True, stop=True, skip_group_check=True,
                )

            def emit_evac(gidx, c0, po, evac_on_act):
                g = glist[gidx]
                if g.outsb is None:
                    g.outsb = osb_pool.tile(
                        [128, n_iters, 256], bf16, tag="osb"
                    )
                dst = g.outsb[:, c0 : c0 + 2, :]
                if evac_on_act:
                    nc.scalar.activation(dst, po, func=Copy, scale=1.0)
                else:
                    nc.vector.tensor_copy(dst, po)
                half = n_iters * 256 // 2
                oq = nc.gpsimd
                if c0 + 1 == n_iters // 2 - 1:
                    oq.dma_start(
                        out=out_d[g.b, g.gi, :, 0:half],
                        in_=g.outsb[:, 0 : n_iters // 2, :],
                    )
                elif c0 + 1 == n_iters - 1:
                    oq.dma_start(
                        out=out_d[g.b, g.gi, :, half : 2 * half],
                        in_=g.outsb[:, n_iters // 2 : n_iters, :],
                    )
                    g.outsb = None
                    g.qt = None
                    g.kv = None

            # ---------------- schedule -----------------------------------
            # chain slot s (0..191): wave w = s//64, r = s%64, c = r//WAVE,
            #   gp = r%WAVE, group g = w*WAVE+gp.
            # act item (g, tb): 4 DR matmuls at slots w*64+16*tb+gp-LAG ...
            #   +3, cast at +4.
            # group kv DMA one wave ahead (spread), qt half a wave ahead.
            events = {}

            def at(slot, fn, *args):
                events.setdefault(slot, []).append((fn, args))

            n_slots = nwaves * slots_per_wave
            for w in range(nwaves):
                for gp in range(WAVE):
                    gidx = w * WAVE + gp
                    if w == 0:
                        # wave 0: kv quarters, just-in-time priority order
                        # (first chain slots gate on the least DMA bytes)
                        at(-50 + gp, kv_part, gidx, 0, 4)
                        at(-46 + gp, kv_part, gidx, 1, 4)
                        at(-42 + gp, qt_part, gidx, 0, 2)
                        at(-38 + gp, kv_part, gidx, 2, 4)
                        at(-34 + gp, qt_part, gidx, 1, 2)
                        at(-30 + gp, kv_part, gidx, 3, 4)
                    else:
                        at(w * 64 - 76 + 8 * gp, kv_part, gidx, 0, 1)
                        at(w * 64 - 72 + 8 * gp, qt_part, gidx, 0, 1)
                    for tb in range(n_tb):
                        t0 = w * 64 + 16 * tb + 4 * gp - LAG
                        for u in range(CB):
                            at(t0 + u, emit_act, gidx, tb, u)
                        at(t0 + CB, emit_cast, gidx, tb)
            for i in range(8):
                at(-58 + i, memset_abct, i)

            # pending out-pairs: (gidx, c0, w0, w1)
            pend = []
            prev_w = [None] * ngroups
            evac_flip = [0]

            lo = min(events)
            for s in range(lo, n_slots + 3):
                for fn, args in events.get(s, ()):
                    fn(*args)
                # interleave within the slot so no two consecutive matmuls
                # target the same PSUM bank, and DVE sees stt BEFORE evac:
                #   out(c0) [pout] .. seed [pg] .. out(c0+1) [pout]
                #   .. grad [pg] + stt .. evac
                po_info = None
                if pend and s >= 6:  # wave-0 warmup: let qt DMA land first
                    gq, c0q, w0q, w1q = pend.pop(0)
                    evac_flip[0] = (evac_flip[0] + 1) % 3
                    on_act = evac_flip[0] != 0  # 2/3 Act, 1/3 DVE
                    po = pout_pool.tile([128, 2, 256], f32, tag="po")
                    emit_out_mm(gq, c0q, w0q, po, 0)
                    po_info = (gq, c0q, w1q, po, on_act)
                in_chain = 0 <= s < n_slots
                if in_chain:
                    w, r = divmod(s, slots_per_wave)
                    c, gp = divmod(r, WAVE)
                    gidx = w * WAVE + gp
                    g = glist[gidx]
                    pg = chain_seed(g, c)
                if po_info is not None:
                    emit_out_mm(po_info[0], po_info[1] + 1, po_info[2],
                                po_info[3], 1)
                if in_chain:
                    chain_grad(g, c, pg)
                    if c % 2 == 1:
                        pend.append((gidx, c - 1, prev_w[gidx], g.wrep))
                    else:
                        prev_w[gidx] = g.wrep
                if po_info is not None:
                    emit_evac(po_info[0], po_info[1], po_info[3], po_info[4])
            while pend:
                gq, c0q, w0q, w1q = pend.pop(0)
                evac_flip[0] = (evac_flip[0] + 1) % 3
                po = pout_pool.tile([128, 2, 256], f32, tag="po")
                emit_out_mm(gq, c0q, w0q, po, 0)
                emit_out_mm(gq, c0q + 1, w1q, po, 1)
                emit_evac(gq, c0q, po, evac_flip[0] != 0)

    _split_excess_waits(nc)
    return nc


def _get_nc():
    if "nc" not in _CACHE:
        _CACHE["nc"] = _build()
    return _CACHE["nc"]


def _host_prep(q, k, v):
    """Host re-layout (token t = c*256 + j*128 + p)."""
    import ml_dtypes

    bf = ml_dtypes.bfloat16
    f8 = ml_dtypes.float8_e4m3
    Bq, Hq, Nq, Dq = q.shape
    hg = Hq // 2
    ni = Nq // 256
    # kv: [b, g, p, c, j, (k0|k1|v0|v1), d]
    k7 = k.reshape(Bq, hg, 2, ni, 2, 128, Dq)
    v7 = (-v).reshape(Bq, hg, 2, ni, 2, 128, Dq)
    kv = np.stack(
        [k7[:, :, 0], k7[:, :, 1], v7[:, :, 0], v7[:, :, 1]], axis=5
    )  # [b, g, c, j, p, 4, d]
    kv = np.ascontiguousarray(
        kv.transpose(0, 1, 4, 2, 3, 5, 6).reshape(Bq, hg, 128, ni * 2 * 4 * Dq)
    ).astype(f8)
    # qt: [b, g, hpair*64+d, c, t]
    q6 = q.reshape(Bq, hg, 2, ni, 256, Dq)
    qt = np.ascontiguousarray(
        q6.transpose(0, 1, 2, 5, 3, 4).reshape(Bq, hg, 128, ni * 256)
    ).astype(bf)
    return kv, qt


def _host_unshuffle(out_host):
    """[b, g, hpair*64+e, c*256+t] bf16 -> (B, N, H*64) f32."""
    Bq, hgq, _, w = out_host.shape
    ni = w // 256
    o6 = np.asarray(out_host, dtype=np.float32).reshape(
        Bq, hgq, 2, 64, ni, 256
    )
    # [b, g, hp, e, c, t] -> [b, c, t, g, hp, e]
    return np.ascontiguousarray(
        o6.transpose(0, 4, 5, 1, 2, 3).reshape(Bq, ni * 256, hgq * 2 * 64)
    )


def kernel(q, k, v, W_init, training=0, return_aux=0, **_unused):
    import ml_dtypes
    from concourse.bass_utils import run_bass_kernel_spmd

    q = np.asarray(q, dtype=np.float32)
    k = np.asarray(k, dtype=np.float32)
    v = np.asarray(v, dtype=np.float32)
    W_init = np.ascontiguousarray(np.asarray(W_init, dtype=np.float32))

    kv, qt = _host_prep(q, k, v)
    wbd = np.zeros((HG, 128, 128), dtype=np.float32)
    wbd[:, 0:64, 0:64] = W_init[0::2]
    wbd[:, 64:128, 64:128] = W_init[1::2]
    wbd = np.ascontiguousarray(
        wbd.transpose(1, 0, 2).reshape(128, HG * 128)
    )
    ident = np.eye(128, dtype=ml_dtypes.bfloat16)

    nc = _get_nc()
    in_maps = []
    for i in range(NCORES):
        sl = slice(i * NB, (i + 1) * NB)
        in_maps.append(
            {"qt": qt[sl], "kv": kv[sl], "W12bd": wbd, "ident": ident}
        )

    trace = bool(int(os.environ.get("BASS_KERNEL_TRACE", "0")))
    res = run_bass_kernel_spmd(
        nc, in_maps, core_ids=list(range(NCORES)), trace=trace
    )
    _CACHE["last_results"] = res
    out_host = np.concatenate(
        [np.asarray(res.results[i]["out"]) for i in range(NCORES)], axis=0
    )
    return _host_unshuffle(out_host)


if __name__ == "__main__":
    rng = np.random.default_rng(0)
    q = rng.standard_normal((B, H, N, D), dtype=np.float32)
    k = rng.standard_normal((B, H, N, D), dtype=np.float32)
    v = rng.standard_normal((B, H, N, D), dtype=np.float32)
    W = (rng.standard_normal((H, D, D)) * D**-0.5).astype(np.float32)
    out = kernel(q, k, v, W)
    print("kernel ran, out shape:", out.shape)


# revision 41
# speedup vs baseline: 1.1406x; 1.0271x over previous
"""Trainium2 Bass kernel for ARM TTT multi-head self-attention (inner-GD scan).

Math per (b, h) pair (B=16, H=12, N=4096, D=64, 16 chunks of m=256 tokens):
    A_i = k_i^T k_i ;  ct_i = k_i^T (-v_i)      (token contraction)
    grad_raw_i = A_i @ W_{i-1} + ct_i
    W_i = W_{i-1} - s * grad_raw_i,  s = 1/(m*D)
    out_i = q_i @ W_i
Pairs are fully independent -> shard B over the 8 NeuronCores (24 chains/core).

v6: v5's measured bottleneck was PE instruction CADENCE (~116ns per matmul
regardless of size: LdWeights + dispatch), 12 matmuls per chunk.  v6 packs
each head-PAIR into block-diagonal 128x128 operands -> 4 matmuls per chunk:

  1. act:  ONE fp8 DoubleRow matmul per chunk contracts all 256 tokens:
           lhsT = [k0|k1] (128t x 2j x 128), rhs = [k0|k1|v0|v1] (x 256)
           -> pac[128, 256]: A0/A1 diag blocks of cols 0:128, ct0/ct1 diag
              blocks of cols 128:256 (junk off-diag).  4 chunks per pac.
  2. cast: per t-block (4 chunks), per pair, ONE activation moves the A/ct
           diag blocks into PERSISTENT pre-zeroed block-diag bf16 tiles
           (abct) - zeros off the diag keep the chain closed in block-diag.
  3. seed: matmul(pg = Id^T @ ctbd)   [start of PSUM accumulation group]
  4. grad: matmul(pg += Abd^T @ Wbd)  [stop]
     stt (DVE, ONE op): Wbd' = -s*pg + Wbd   (off-diag stays 0: 0*s+0)
  5. out:  matmul(pout = Wbd'(lhsT) @ qt[128 dpair x 256 tok]) - both heads
           in one 256-col matmul; emitted in chunk-PAIRS, one evac per pair.

The serial W-chain round trip (PE->DVE->PE ~1us) is hidden by round-robining
chunks across a 4-group window; acts/casts lead by LAG slots; outs trail.
PSUM budget (8 banks): pac 2x2 + pg 2 + pout 2.
GpSimd CANNOT access PSUM on TRN2, so casts go to Act and evacs to DVE/Act.

Device layouts (token t = c*256 + j*128 + p):
    kv (per group):  [128(p), 16(c), 2(j), 4(k0|k1|v0|v1), 64]  fp8 (v negated)
    qt (per group):  [128(hpair*64+d), 16(c), 256(t=j*128+p)]   bf16
    out (per group): [128(hpair*64+e), 16(c), 256(t)]           bf16
    W12bd: [hg, 128, 128] f32 block-diag(W_h0, W_h1); carried chain is bf16.
"""

import os
import sys

sys.path.insert(0, "/opt/trn_rl_repo")

import numpy as np

B, H, N, D = 16, 12, 4096, 64
N_ITERS = 16
M = N // N_ITERS  # 256 tokens per chunk
NCORES = 8
NB = B // NCORES  # batches per core
HG = H // 2  # head-pair groups per batch
SCALE = 1.0 / (M * D)
WAVE = 4  # chain interleave width (groups round-robined per chunk)
LAG = 14  # slots the act/cast stream leads the chain stream
CB = 4  # chunks per t-block (pac granularity)
USE_DR = True  # fp8 DoubleRow: one act matmul per chunk (else 2, j-accum)

_CACHE = {}


def _split_excess_waits(nc):
    """walrus in this env accepts at most ONE sem wait per instruction
    (two on EventSemaphore); move excess waits onto EventSemaphore
    instructions inserted just before on the same engine."""
    import concourse.mybir as mybir

    n_ev = 0
    for f in nc.m.functions:
        for b in f.blocks:
            il = b.instructions
            idx = 0
            while idx < len(il):
                inst = il[idx]
                si = getattr(inst, "sync_info", None)
                if si is not None and len(si.on_wait) > 1:
                    waits = list(si.on_wait)
                    si.on_wait = [waits[0]]
                    extra = waits[1:]
                    for g in range(0, len(extra), 2):
                        n_ev += 1
                        ev = mybir.InstEventSemaphore(
                            name=f"EVSPLIT-{n_ev}",
                            engine=inst.engine,
                            ins=[],
                            outs=[],
                            sync_info=mybir.SyncInfo(
                                on_wait=extra[g : g + 2], on_update=[]
                            ),
                        )
                        nc.register_instruction(ev)
                        il.insert(idx, ev)
                        idx += 1
                idx += 1
    return n_ev


class _G:
    __slots__ = ("kv", "qt", "outsb", "wrep", "abct", "pac", "b", "gi")


def _build(nb=NB, hg=HG, n_iters=N_ITERS):
    import concourse.bass as bass
    import concourse.mybir as mybir
    from concourse.tile import TileContext

    f32 = mybir.dt.float32
    bf16 = mybir.dt.bfloat16
    fp8 = mybir.dt.float8e4
    Copy = mybir.ActivationFunctionType.Copy
    mult = mybir.AluOpType.mult
    add = mybir.AluOpType.add
    DR = mybir.MatmulPerfMode.DoubleRow

    ngroups = nb * hg  # 12
    nwaves = ngroups // WAVE  # 3
    slots_per_wave = WAVE * n_iters  # 64
    n_tb = n_iters // CB  # 4 t-blocks per group

    nc = bass.Bass()
    q_d = nc.declare_dram_parameter(
        "qt", [nb, hg, 128, n_iters * 256], bf16, isOutput=False
    )
    kv_d = nc.declare_dram_parameter(
        "kv", [nb, hg, 128, n_iters * 2 * 4 * D], fp8, isOutput=False
    )
    w_d = nc.declare_dram_parameter("W12bd", [128, hg * 128], f32, isOutput=False)
    id_d = nc.declare_dram_parameter("ident", [128, 128], bf16, isOutput=False)
    out_d = nc.declare_dram_parameter(
        "out", [nb, hg, 128, n_iters * 256], bf16, isOutput=True
    )

    with TileContext(nc) as tc:
        with (
            tc.tile_pool(name="singles", bufs=1) as singles,
            tc.tile_pool(name="kv", bufs=8) as kv_pool,
            tc.tile_pool(name="qt", bufs=7) as qt_pool,
            tc.tile_pool(name="osb", bufs=5) as osb_pool,
            tc.tile_pool(name="abct", bufs=8) as abct_pool,
            tc.tile_pool(name="wrp", bufs=14) as wrp_pool,
            tc.tile_pool(name="pac", bufs=2, space="PSUM") as pac_pool,
            tc.tile_pool(name="pg", bufs=2, space="PSUM") as pg_pool,
            tc.tile_pool(name="pout", bufs=2, space="PSUM") as pout_pool,
        ):
            ident = singles.tile([128, 128], bf16)
            nc.sync.dma_start(out=ident, in_=id_d[:, :])
            # hoist the lazy ACT_TABLE_LOAD off the first real cast's path
            dummy = singles.tile([128, 1], bf16)
            nc.scalar.activation(
                dummy, ident[:, 0:1], func=Copy, scale=1.0
            )
            winit = singles.tile([128, hg, 128], f32)
            winit_dma = [False]

            def load_winit():
                winit_dma[0] = True
                nc.sync.dma_start(
                    out=winit, in_=w_d.rearrange("p (g e) -> p g e", g=hg)
                )

            # persistent abct rotation: casts only ever write the diag
            # blocks, so the one-time memset zeros persist across reuses
            # (same logical tensors, manual rotation).  The memsets are
            # emitted from the schedule (after wave-0 DMA triggers) so they
            # don't block the GpSimd DMA queue at startup.
            abct_tiles = []
            for _i in range(8):
                abt = abct_pool.tile([128, CB, 2, 128], bf16, tag="abct")
                abct_tiles.append(abt)
            abct_ctr = [0]

            def memset_abct(i):
                nc.gpsimd.memset(abct_tiles[i], 0.0)

            glist = [None] * ngroups

            def ensure_group(gidx):
                if glist[gidx] is not None:
                    return
                g = _G()
                g.b, g.gi = divmod(gidx, hg)
                g.wrep = None
                g.abct = {}
                g.pac = None
                g.kv = None
                g.qt = None
                g.outsb = None
                glist[gidx] = g

            def init_wrep(gidx):
                # emitted strictly after load_winit so the copy waits on it
                ensure_group(gidx)
                g = glist[gidx]
                g.wrep = wrp_pool.tile([128, 128], bf16, tag="wrep")
                nc.vector.tensor_copy(g.wrep, winit[:, g.gi, :])

            def _q(gidx):
                # single queue: transfers fair-share DMA engines, so issue
                # order IS the priority order - keep inputs on one queue
                return nc.sync

            def kv_part(gidx, h, nh):
                # kv DMA in 1/nh fractions (latency vs per-transfer overhead)
                ensure_group(gidx)
                g = glist[gidx]
                if g.kv is None:
                    g.kv = kv_pool.tile(
                        [128, n_iters, 2, 4, D], fp8, tag="kv"
                    )
                hc = n_iters // nh
                w2 = hc * 2 * 4 * D
                _q(gidx).dma_start(
                    out=g.kv[:, h * hc : (h + 1) * hc, :, :, :],
                    in_=kv_d[g.b, g.gi, :, h * w2 : (h + 1) * w2].rearrange(
                        "p (c j s d) -> p c j s d", j=2, s=4, d=D
                    ),
                )

            def qt_part(gidx, h, nh):
                g = glist[gidx]
                if g.qt is None:
                    g.qt = qt_pool.tile([128, n_iters, 256], bf16, tag="qt")
                hc = n_iters // nh
                w2 = hc * 256
                _q(gidx).dma_start(
                    out=g.qt[:, h * hc : (h + 1) * hc, :],
                    in_=q_d[g.b, g.gi, :, h * w2 : (h + 1) * w2].rearrange(
                        "p (c t) -> p c t", t=256
                    ),
                )

            def emit_act(gidx, tb, u):
                # chunk c = CB*tb + u Gram matmul into pac[:, u, :, :]
                g = glist[gidx]
                c = CB * tb + u
                if u == 0:
                    g.pac = pac_pool.tile([128, CB, 2, 128], f32, tag="pac")
                if USE_DR:
                    nc.tensor.matmul(
                        g.pac[:, u, :, :],
                        lhsT=g.kv[:, c, :, 0:2, :],
                        rhs=g.kv[:, c, :, :, :],
                        start=True, stop=True,
                        perf_mode=DR,
                        skip_group_check=True,
                    )
                else:
                    for j in (0, 1):
                        nc.tensor.matmul(
                            g.pac[:, u, :, :],
                            lhsT=g.kv[:, c, j, 0:2, :],
                            rhs=g.kv[:, c, j, :, :],
                            start=(j == 0), stop=(j == 1),
                            skip_group_check=True,
                        )

            def emit_cast(gidx, tb):
                # A/ct diag blocks -> block-diag bf16 (abct off-diag stays 0)
                g = glist[gidx]
                ab = abct_tiles[abct_ctr[0] % len(abct_tiles)]
                on_dve = abct_ctr[0] % 4 == 0
                abct_ctr[0] += 1
                nc.scalar.activation(
                    ab[0:64, :, :, 0:64], g.pac[0:64, :, :, 0:64],
                    func=Copy, scale=1.0,
                )
                if on_dve:
                    nc.vector.tensor_copy(
                        ab[64:128, :, :, 64:128], g.pac[64:128, :, :, 64:128]
                    )
                else:
                    nc.scalar.activation(
                        ab[64:128, :, :, 64:128], g.pac[64:128, :, :, 64:128],
                        func=Copy, scale=1.0,
                    )
                g.abct[tb] = ab
                g.pac = None

            def chain_seed(g, c):
                tb, u = divmod(c, CB)
                ab = g.abct[tb]
                pg = pg_pool.tile([128, 512], f32, tag="pg")
                nc.tensor.matmul(
                    pg[:, 0:128],
                    lhsT=ident[:, :],
                    rhs=ab[:, u, 1, :],
                    start=True, stop=False, skip_group_check=True,
                )
                return pg

            def chain_grad(g, c, pg):
                tb, u = divmod(c, CB)
                ab = g.abct[tb]
                nc.tensor.matmul(
                    pg[:, 0:128],
                    lhsT=ab[:, u, 0, :],
                    rhs=g.wrep[:, :],
                    start=False, stop=True, skip_group_check=True,
                )
                wnew = wrp_pool.tile([128, 128], bf16, tag="wrep")
                nc.vector.scalar_tensor_tensor(
                    wnew, pg[:, 0:128], -SCALE, g.wrep,
                    op0=mult, op1=add,
                )
                g.wrep = wnew
                if u == CB - 1:
                    del g.abct[tb]

            def emit_out_mm(gidx, c, wrep, po, slot_idx):
                g = glist[gidx]
                nc.tensor.matmul(
                    po[:, slot_idx, :], lhsT=wrep[:, :], rhs=g.qt[:, c, :],
                    start=True, stop=True, skip_group_check=True,
                )

            def emit_evac(gidx, c0, po, evac_on_act):
                g = glist[gidx]
                if g.outsb is None:
                    g.outsb = osb_pool.tile(
                        [128, n_iters, 256], bf16, tag="osb"
                    )
                dst = g.outsb[:, c0 : c0 + 2, :]
                if evac_on_act:
                    nc.scalar.activation(dst, po, func=Copy, scale=1.0)
                else:
                    nc.vector.tensor_copy(dst, po)
                half = n_iters * 256 // 2
                if gidx >= ngroups - WAVE and c0 + 1 == n_iters - 1:
                    # last wave: spread the tail DMAs over idle queues
                    oq = (nc.gpsimd, nc.sync, nc.scalar, nc.gpsimd)[gidx % 4]
                else:
                    oq = nc.gpsimd
                if c0 + 1 == n_iters // 2 - 1:
                    oq.dma_start(
                        out=out_d[g.b, g.gi, :, 0:half],
                        in_=g.outsb[:, 0 : n_iters // 2, :],
                    )
                elif c0 + 1 == n_iters - 1:
                    oq.dma_start(
                        out=out_d[g.b, g.gi, :, half : 2 * half],
                        in_=g.outsb[:, n_iters // 2 : n_iters, :],
                    )
                    g.outsb = None
                    g.qt = None
                    g.kv = None

            # ---------------- schedule -----------------------------------
            # chain slot s (0..191): wave w = s//64, r = s%64, c = r//WAVE,
            #   gp = r%WAVE, group g = w*WAVE+gp.
            # act item (g, tb): 4 DR matmuls at slots w*64+16*tb+gp-LAG ...
            #   +3, cast at +4.
            # group kv DMA one wave ahead (spread), qt half a wave ahead.
            events = {}

            def at(slot, fn, *args):
                events.setdefault(slot, []).append((fn, args))

            n_slots = nwaves * slots_per_wave
            for w in range(nwaves):
                for gp in range(WAVE):
                    gidx = w * WAVE + gp
                    if w == 0:
                        # wave 0: kv quarters, just-in-time priority order
                        # (first chain slots gate on the least DMA bytes)
                        at(-50 + gp, kv_part, gidx, 0, 4)
                        at(-45 + gp, kv_part, gidx, 1, 4)
                        at(-40 + gp, qt_part, gidx, 0, 2)
                        at(-36 + gp, kv_part, gidx, 2, 4)
                        at(-32 + gp, qt_part, gidx, 1, 2)
                        at(-28 + gp, kv_part, gidx, 3, 4)
                    else:
                        # kv one wave early (highest priority after wave 0);
                        # qt later - first needed only ~6 slots into a wave
                        at(w * 64 - 88 + 4 * gp, kv_part, gidx, 0, 1)
                        at(w * 64 - 20 + 4 * gp, qt_part, gidx, 0, 1)
                    at(w * 64 - 43 + gp if w == 0 else w * 64 - 86 + 4 * gp,
                       init_wrep, gidx)
                    for tb in range(n_tb):
                        t0 = w * 64 + 16 * tb + 4 * gp - LAG
                        for u in range(CB):
                            at(t0 + u, emit_act, gidx, tb, u)
                        at(t0 + CB, emit_cast, gidx, tb)
            at(-46, load_winit)
            for i in range(8):
                at(-58 + i, memset_abct, i)

            # pending out-pairs: (gidx, c0, w0, w1)
            pend = []
            prev_w = [None] * ngroups
            evac_flip = [0]

            lo = min(events)
            for s in range(lo, n_slots + 3):
                for fn, args in events.get(s, ()):
                    fn(*args)
                # interleave within the slot so no two consecutive matmuls
                # target the same PSUM bank, and DVE sees stt BEFORE evac:
                #   out(c0) [pout] .. seed [pg] .. out(c0+1) [pout]
                #   .. grad [pg] + stt .. evac
                po_info = None
                if pend and s >= 6:  # wave-0 warmup: let qt DMA land first
                    gq, c0q, w0q, w1q = pend.pop(0)
                    evac_flip[0] = (evac_flip[0] + 1) % 3
                    on_act = evac_flip[0] != 0  # 2/3 Act, 1/3 DVE
                    po = pout_pool.tile([128, 2, 256], f32, tag="po")
                    emit_out_mm(gq, c0q, w0q, po, 0)
                    po_info = (gq, c0q, w1q, po, on_act)
                in_chain = 0 <= s < n_slots
                if in_chain:
                    w, r = divmod(s, slots_per_wave)
                    c, gp = divmod(r, WAVE)
                    gidx = w * WAVE + gp
                    g = glist[gidx]
                    pg = chain_seed(g, c)
                if po_info is not None:
                    emit_out_mm(po_info[0], po_info[1] + 1, po_info[2],
                                po_info[3], 1)
                if in_chain:
                    chain_grad(g, c, pg)
                    if c % 2 == 1:
                        pend.append((gidx, c - 1, prev_w[gidx], g.wrep))
                    else:
                        prev_w[gidx] = g.wrep
                if po_info is not None:
                    emit_evac(po_info[0], po_info[1], po_info[3], po_info[4])
            while pend:
                gq, c0q, w0q, w1q = pend.pop(0)
                evac_flip[0] = (evac_flip[0] + 1) % 3
                po = pout_pool.tile([128, 2, 256], f32, tag="po")
                emit_out_mm(gq, c0q, w0q, po, 0)
                emit_out_mm(gq, c0q + 1, w1q, po, 1)
                emit_evac(gq, c0q, po, evac_flip[0] != 0)

    _split_excess_waits(nc)
    return nc


def _get_nc():
    if "nc" not in _CACHE:
        _CACHE["nc"] = _build()
    return _CACHE["nc"]


def _host_prep(q, k, v):
    """Host re-layout (token t = c*256 + j*128 + p)."""
    import ml_dtypes

    bf = ml_dtypes.bfloat16
    f8 = ml_dtypes.float8_e4m3
    Bq, Hq, Nq, Dq = q.shape
    hg = Hq // 2
    ni = Nq // 256
    # kv: [b, g, p, c, j, (k0|k1|v0|v1), d]
    k7 = k.reshape(Bq, hg, 2, ni, 2, 128, Dq)
    v7 = (-v).reshape(Bq, hg, 2, ni, 2, 128, Dq)
    kv = np.stack(
        [k7[:, :, 0], k7[:, :, 1], v7[:, :, 0], v7[:, :, 1]], axis=5
    )  # [b, g, c, j, p, 4, d]
    kv = np.ascontiguousarray(
        kv.transpose(0, 1, 4, 2, 3, 5, 6).reshape(Bq, hg, 128, ni * 2 * 4 * Dq)
    ).astype(f8)
    # qt: [b, g, hpair*64+d, c, t]
    q6 = q.reshape(Bq, hg, 2, ni, 256, Dq)
    qt = np.ascontiguousarray(
        q6.transpose(0, 1, 2, 5, 3, 4).reshape(Bq, hg, 128, ni * 256)
    ).astype(bf)
    return kv, qt


def _host_unshuffle(out_host):
    """[b, g, hpair*64+e, c*256+t] bf16 -> (B, N, H*64) f32."""
    Bq, hgq, _, w = out_host.shape
    ni = w // 256
    o6 = np.asarray(out_host, dtype=np.float32).reshape(
        Bq, hgq, 2, 64, ni, 256
    )
    # [b, g, hp, e, c, t] -> [b, c, t, g, hp, e]
    return np.ascontiguousarray(
        o6.transpose(0, 4, 5, 1, 2, 3).reshape(Bq, ni * 256, hgq * 2 * 64)
    )


def kernel(q, k, v, W_init, training=0, return_aux=0, **_unused):
    import ml_dtypes
    from concourse.bass_utils import run_bass_kernel_spmd

    q = np.asarray(q, dtype=np.float32)
    k = np.asarray(k, dtype=np.float32)
    v = np.asarray(v, dtype=np.float32)
    W_init = np.ascontiguousarray(np.asarray(W_init, dtype=np.float32))

    kv, qt = _host_prep(q, k, v)
    wbd = np.zeros((HG, 128, 128), dtype=np.float32)
    wbd[:, 0:64, 0:64] = W_init[0::2]
    wbd[:, 64:128, 64:128] = W_init[1::2]
    wbd = np.ascontiguousarray(
        wbd.transpose(1, 0, 2).reshape(128, HG * 128)
    )
    ident = np.eye(128, dtype=ml_dtypes.bfloat16)

    nc = _get_nc()
    in_maps = []
    for i in range(NCORES):
        sl = slice(i * NB, (i + 1) * NB)
        in_maps.append(
            {"qt": qt[sl], "kv": kv[sl], "W12bd": wbd, "ident": ident}
        )

    trace = bool(int(os.environ.get("BASS_KERNEL_TRACE", "0")))
    res = run_bass_kernel_spmd(
        nc, in_maps, core_ids=list(range(NCORES)), trace=trace
    )
    _CACHE["last_results"] = res
    out_host = np.concatenate(
        [np.asarray(res.results[i]["out"]) for i in range(NCORES)], axis=0
    )
    return _host_unshuffle(out_host)


if __name__ == "__main__":
    rng = np.random.default_rng(0)
    q = rng.standard_normal((B, H, N, D), dtype=np.float32)
    k = rng.standard_normal((B, H, N, D), dtype=np.float32)
    v = rng.standard_normal((B, H, N, D), dtype=np.float32)
    W = (rng.standard_normal((H, D, D)) * D**-0.5).astype(np.float32)
    out = kernel(q, k, v, W)
    print("kernel ran, out shape:", out.shape)


# revision 49
# speedup vs baseline: 1.1689x; 1.0248x over previous
"""Trainium2 Bass kernel for ARM TTT multi-head self-attention (inner-GD scan).

Math per (b, h) pair (B=16, H=12, N=4096, D=64, 16 chunks of m=256 tokens):
    A_i = k_i^T k_i ;  ct_i = k_i^T (-v_i)      (token contraction)
    grad_raw_i = A_i @ W_{i-1} + ct_i
    W_i = W_{i-1} - s * grad_raw_i,  s = 1/(m*D)
    out_i = q_i @ W_i
Pairs are fully independent -> shard B over the 8 NeuronCores (24 chains/core).

v6: v5's measured bottleneck was PE instruction CADENCE (~116ns per matmul
regardless of size: LdWeights + dispatch), 12 matmuls per chunk.  v6 packs
each head-PAIR into block-diagonal 128x128 operands -> 4 matmuls per chunk:

  1. act:  ONE fp8 DoubleRow matmul per chunk contracts all 256 tokens:
           lhsT = [k0|k1] (128t x 2j x 128), rhs = [k0|k1|v0|v1] (x 256)
           -> pac[128, 256]: A0/A1 diag blocks of cols 0:128, ct0/ct1 diag
              blocks of cols 128:256 (junk off-diag).  4 chunks per pac.
  2. cast: per t-block (4 chunks), per pair, ONE activation moves the A/ct
           diag blocks into PERSISTENT pre-zeroed block-diag bf16 tiles
           (abct) - zeros off the diag keep the chain closed in block-diag.
  3. seed: matmul(pg = Id^T @ ctbd)   [start of PSUM accumulation group]
  4. grad: matmul(pg += Abd^T @ Wbd)  [stop]
     stt (DVE, ONE op): Wbd' = -s*pg + Wbd   (off-diag stays 0: 0*s+0)
  5. out:  matmul(pout = Wbd'(lhsT) @ qt[128 dpair x 256 tok]) - both heads
           in one 256-col matmul; emitted in chunk-PAIRS, one evac per pair.

The serial W-chain round trip (PE->DVE->PE ~1us) is hidden by round-robining
chunks across a 4-group window; acts/casts lead by LAG slots; outs trail.
PSUM budget (8 banks): pac 2x2 + pg 2 + pout 2.
GpSimd CANNOT access PSUM on TRN2, so casts go to Act and evacs to DVE/Act.

Device layouts (token t = c*256 + j*128 + p):
    kv (per group):  [128(p), 16(c), 2(j), 4(k0|k1|v0|v1), 64]  fp8 (v negated)
    qt (per group):  [128(hpair*64+d), 16(c), 256(t=j*128+p)]   bf16
    out (per group): [128(hpair*64+e), 16(c), 256(t)]           bf16
    W12bd: [hg, 128, 128] f32 block-diag(W_h0, W_h1); carried chain is bf16.
"""

import os
import sys

sys.path.insert(0, "/opt/trn_rl_repo")

import numpy as np

B, H, N, D = 16, 12, 4096, 64
N_ITERS = 16
M = N // N_ITERS  # 256 tokens per chunk
NCORES = 8
NB = B // NCORES  # batches per core
HG = H // 2  # head-pair groups per batch
SCALE = 1.0 / (M * D)
WAVE = 4  # chain interleave width (groups round-robined per chunk)
LAG = 14  # slots the act/cast stream leads the chain stream
CB = 4  # chunks per t-block (pac granularity)
USE_DR = True  # fp8 DoubleRow: one act matmul per chunk (else 2, j-accum)
DEFER_OUT_DMA = False

_CACHE = {}


def _split_excess_waits(nc):
    """walrus in this env accepts at most ONE sem wait per instruction
    (two on EventSemaphore); move excess waits onto EventSemaphore
    instructions inserted just before on the same engine."""
    import concourse.mybir as mybir

    n_ev = 0
    for f in nc.m.functions:
        for b in f.blocks:
            il = b.instructions
            idx = 0
            while idx < len(il):
                inst = il[idx]
                si = getattr(inst, "sync_info", None)
                if si is not None and len(si.on_wait) > 1:
                    waits = list(si.on_wait)
                    si.on_wait = [waits[0]]
                    extra = waits[1:]
                    for g in range(0, len(extra), 2):
                        n_ev += 1
                        ev = mybir.InstEventSemaphore(
                            name=f"EVSPLIT-{n_ev}",
                            engine=inst.engine,
                            ins=[],
                            outs=[],
                            sync_info=mybir.SyncInfo(
                                on_wait=extra[g : g + 2], on_update=[]
                            ),
                        )
                        nc.register_instruction(ev)
                        il.insert(idx, ev)
                        idx += 1
                idx += 1
    return n_ev


class _G:
    __slots__ = ("kv", "qt", "outsb", "wrep", "abct", "pac", "b", "gi")


def _build(nb=NB, hg=HG, n_iters=N_ITERS):
    import concourse.bass as bass
    import concourse.mybir as mybir
    from concourse.tile import TileContext

    f32 = mybir.dt.float32
    bf16 = mybir.dt.bfloat16
    fp8 = mybir.dt.float8e4
    Copy = mybir.ActivationFunctionType.Copy
    mult = mybir.AluOpType.mult
    add = mybir.AluOpType.add
    DR = mybir.MatmulPerfMode.DoubleRow

    ngroups = nb * hg  # 12
    nwaves = ngroups // WAVE  # 3
    slots_per_wave = WAVE * n_iters  # 64
    n_tb = n_iters // CB  # 4 t-blocks per group

    nc = bass.Bass()
    q_d = nc.declare_dram_parameter(
        "qt", [nb, hg, 128, n_iters * 256], bf16, isOutput=False
    )
    kv_d = nc.declare_dram_parameter(
        "kv", [nb, hg, 128, n_iters * 2 * 4 * D], fp8, isOutput=False
    )
    w_d = nc.declare_dram_parameter("W12bd", [128, hg * 128], f32, isOutput=False)
    id_d = nc.declare_dram_parameter("ident", [128, 128], bf16, isOutput=False)
    out_d = nc.declare_dram_parameter(
        "out", [nb, hg, 128, n_iters * 256], bf16, isOutput=True
    )

    with TileContext(nc) as tc:
        with (
            tc.tile_pool(name="singles", bufs=1) as singles,
            tc.tile_pool(name="kv", bufs=8) as kv_pool,
            tc.tile_pool(name="qt", bufs=6) as qt_pool,
            tc.tile_pool(name="osb", bufs=6) as osb_pool,
            tc.tile_pool(name="abct", bufs=8) as abct_pool,
            tc.tile_pool(name="wrp", bufs=14) as wrp_pool,
            tc.tile_pool(name="pac", bufs=2, space="PSUM") as pac_pool,
            tc.tile_pool(name="pg", bufs=2, space="PSUM") as pg_pool,
            tc.tile_pool(name="pout", bufs=2, space="PSUM") as pout_pool,
        ):
            ident = singles.tile([128, 128], bf16)
            nc.sync.dma_start(out=ident, in_=id_d[:, :])
            # hoist the lazy ACT_TABLE_LOAD off the first real cast's path
            dummy = singles.tile([128, 1], bf16)
            nc.scalar.activation(
                dummy, ident[:, 0:1], func=Copy, scale=1.0
            )
            winit = singles.tile([128, hg, 128], f32)
            winit_dma = [False]

            def load_winit():
                winit_dma[0] = True
                nc.sync.dma_start(
                    out=winit, in_=w_d.rearrange("p (g e) -> p g e", g=hg)
                )

            # persistent abct rotation: casts only ever write the diag
            # blocks, so the one-time memset zeros persist across reuses
            # (same logical tensors, manual rotation).  The memsets are
            # emitted from the schedule (after wave-0 DMA triggers) so they
            # don't block the GpSimd DMA queue at startup.
            abct_tiles = []
            for _i in range(8):
                abt = abct_pool.tile([128, CB, 2, 128], bf16, tag="abct")
                abct_tiles.append(abt)
            abct_ctr = [0]

            def memset_abct(i):
                nc.gpsimd.memset(abct_tiles[i], 0.0)

            glist = [None] * ngroups

            def ensure_group(gidx):
                if glist[gidx] is not None:
                    return
                g = _G()
                g.b, g.gi = divmod(gidx, hg)
                g.wrep = None
                g.abct = {}
                g.pac = None
                g.kv = None
                g.qt = None
                g.outsb = None
                glist[gidx] = g

            def init_wrep(gidx):
                # emitted strictly after load_winit so the copy waits on it
                ensure_group(gidx)
                g = glist[gidx]
                g.wrep = wrp_pool.tile([128, 128], bf16, tag="wrep")
                nc.vector.tensor_copy(g.wrep, winit[:, g.gi, :])

            def _q(gidx):
                # single queue: transfers fair-share DMA engines, so issue
                # order IS the priority order - keep inputs on one queue
                return nc.sync

            def kv_part(gidx, h, nh):
                # kv DMA in 1/nh fractions (latency vs per-transfer overhead)
                ensure_group(gidx)
                g = glist[gidx]
                if g.kv is None:
                    g.kv = kv_pool.tile(
                        [128, n_iters, 2, 4, D], fp8, tag="kv"
                    )
                hc = n_iters // nh
                w2 = hc * 2 * 4 * D
                _q(gidx).dma_start(
                    out=g.kv[:, h * hc : (h + 1) * hc, :, :, :],
                    in_=kv_d[g.b, g.gi, :, h * w2 : (h + 1) * w2].rearrange(
                        "p (c j s d) -> p c j s d", j=2, s=4, d=D
                    ),
                )

            def qt_part(gidx, h, nh):
                g = glist[gidx]
                if g.qt is None:
                    g.qt = qt_pool.tile([128, n_iters, 256], bf16, tag="qt")
                hc = n_iters // nh
                w2 = hc * 256
                _q(gidx).dma_start(
                    out=g.qt[:, h * hc : (h + 1) * hc, :],
                    in_=q_d[g.b, g.gi, :, h * w2 : (h + 1) * w2].rearrange(
                        "p (c t) -> p c t", t=256
                    ),
                )

            def emit_act(gidx, tb, u):
                # chunk c = CB*tb + u Gram matmul into pac[:, u, :, :]
                g = glist[gidx]
                c = CB * tb + u
                if u == 0:
                    g.pac = pac_pool.tile([128, CB, 2, 128], f32, tag="pac")
                if USE_DR:
                    nc.tensor.matmul(
                        g.pac[:, u, :, :],
                        lhsT=g.kv[:, c, :, 0:2, :],
                        rhs=g.kv[:, c, :, :, :],
                        start=True, stop=True,
                        perf_mode=DR,
                        skip_group_check=True,
                    )
                else:
                    for j in (0, 1):
                        nc.tensor.matmul(
                            g.pac[:, u, :, :],
                            lhsT=g.kv[:, c, j, 0:2, :],
                            rhs=g.kv[:, c, j, :, :],
                            start=(j == 0), stop=(j == 1),
                            skip_group_check=True,
                        )

            def emit_cast(gidx, tb):
                # A/ct diag blocks -> block-diag bf16 (abct off-diag stays 0)
                g = glist[gidx]
                ab = abct_tiles[abct_ctr[0] % len(abct_tiles)]
                on_dve = abct_ctr[0] % 4 == 0
                abct_ctr[0] += 1
                nc.scalar.activation(
                    ab[0:64, :, :, 0:64], g.pac[0:64, :, :, 0:64],
                    func=Copy, scale=1.0,
                )
                if on_dve:
                    nc.vector.tensor_copy(
                        ab[64:128, :, :, 64:128], g.pac[64:128, :, :, 64:128]
                    )
                else:
                    nc.scalar.activation(
                        ab[64:128, :, :, 64:128], g.pac[64:128, :, :, 64:128],
                        func=Copy, scale=1.0,
                    )
                g.abct[tb] = ab
                g.pac = None

            def chain_seed(g, c):
                tb, u = divmod(c, CB)
                ab = g.abct[tb]
                pg = pg_pool.tile([128, 512], f32, tag="pg")
                nc.tensor.matmul(
                    pg[:, 0:128],
                    lhsT=ident[:, :],
                    rhs=ab[:, u, 1, :],
                    start=True, stop=False, skip_group_check=True,
                )
                return pg

            def chain_grad(g, c, pg):
                tb, u = divmod(c, CB)
                ab = g.abct[tb]
                nc.tensor.matmul(
                    pg[:, 0:128],
                    lhsT=ab[:, u, 0, :],
                    rhs=g.wrep[:, :],
                    start=False, stop=True, skip_group_check=True,
                )
                wnew = wrp_pool.tile([128, 128], bf16, tag="wrep")
                nc.vector.scalar_tensor_tensor(
                    wnew, pg[:, 0:128], -SCALE, g.wrep,
                    op0=mult, op1=add,
                )
                g.wrep = wnew
                if u == CB - 1:
                    del g.abct[tb]

            def emit_out_mm(gidx, c, wrep, po, slot_idx):
                g = glist[gidx]
                nc.tensor.matmul(
                    po[:, slot_idx, :], lhsT=wrep[:, :], rhs=g.qt[:, c, :],
                    start=True, stop=True, skip_group_check=True,
                )

            def emit_evac(gidx, c0, po, evac_on_act):
                g = glist[gidx]
                if g.outsb is None:
                    g.outsb = osb_pool.tile(
                        [128, n_iters, 256], bf16, tag="osb"
                    )
                dst = g.outsb[:, c0 : c0 + 2, :]
                if evac_on_act:
                    nc.scalar.activation(dst, po, func=Copy, scale=1.0)
                else:
                    nc.vector.tensor_copy(dst, po)
                half = n_iters * 256 // 2
                quart = half // 2
                last_wave = gidx >= ngroups - WAVE
                if last_wave:
                    # last wave: drain in quarters over the idle queues so
                    # the epilogue tail is short
                    oq = (nc.gpsimd, nc.sync, nc.scalar, nc.gpsimd)[gidx % 4]
                    if c0 + 1 == n_iters // 2 - 1:
                        oq.dma_start(
                            out=out_d[g.b, g.gi, :, 0:half],
                            in_=g.outsb[:, 0 : n_iters // 2, :],
                        )
                    elif c0 + 1 == 3 * n_iters // 4 - 1:
                        oq.dma_start(
                            out=out_d[g.b, g.gi, :, half : half + quart],
                            in_=g.outsb[:, n_iters // 2 : 3 * n_iters // 4, :],
                        )
                    elif c0 + 1 == n_iters - 1:
                        oq.dma_start(
                            out=out_d[g.b, g.gi, :, half + quart :],
                            in_=g.outsb[:, 3 * n_iters // 4 :, :],
                        )
                        g.outsb = None
                        g.qt = None
                        g.kv = None
                    return
                if c0 + 1 == n_iters // 2 - 1:
                    nc.gpsimd.dma_start(
                        out=out_d[g.b, g.gi, :, 0:half],
                        in_=g.outsb[:, 0 : n_iters // 2, :],
                    )
                elif c0 + 1 == n_iters - 1:
                    if DEFER_OUT_DMA:
                        # defer the 2nd-half DMA past the wave boundary so
                        # it doesn't steal HBM from the next wave's kv/qt
                        defer_out.append((gidx, g.outsb))
                    else:
                        nc.gpsimd.dma_start(
                            out=out_d[g.b, g.gi, :, half : 2 * half],
                            in_=g.outsb[:, n_iters // 2 : n_iters, :],
                        )
                    g.outsb = None
                    g.qt = None
                    g.kv = None

            # ---------------- schedule -----------------------------------
            # chain slot s (0..191): wave w = s//64, r = s%64, c = r//WAVE,
            #   gp = r%WAVE, group g = w*WAVE+gp.
            # act item (g, tb): 4 DR matmuls at slots w*64+16*tb+gp-LAG ...
            #   +3, cast at +4.
            # group kv DMA one wave ahead (spread), qt half a wave ahead.
            events = {}

            def at(slot, fn, *args):
                events.setdefault(slot, []).append((fn, args))

            n_slots = nwaves * slots_per_wave
            for w in range(nwaves):
                for gp in range(WAVE):
                    gidx = w * WAVE + gp
                    if w == 0:
                        # wave 0: kv quarters, just-in-time priority order
                        # (first chain slots gate on the least DMA bytes)
                        at(-50 + gp, kv_part, gidx, 0, 4)
                        at(-45 + gp, kv_part, gidx, 1, 4)
                        at(-40 + gp, qt_part, gidx, 0, 2)
                        at(-36 + gp, kv_part, gidx, 2, 4)
                        at(-32 + gp, qt_part, gidx, 1, 2)
                        at(-28 + gp, kv_part, gidx, 3, 4)
                    else:
                        # kv one wave early (highest priority after wave 0);
                        # qt later - first needed only ~6 slots into a wave
                        at(w * 64 - 88 + 4 * gp, kv_part, gidx, 0, 1)
                        at(w * 64 - 71 + 4 * gp, qt_part, gidx, 0, 1)
                    at(w * 64 - 43 + gp if w == 0 else w * 64 - 86 + 4 * gp,
                       init_wrep, gidx)
                    for tb in range(n_tb):
                        t0 = w * 64 + 16 * tb + 4 * gp - LAG
                        for u in range(CB):
                            at(t0 + u, emit_act, gidx, tb, u)
                        at(t0 + CB, emit_cast, gidx, tb)
            at(-46, load_winit)
            for i in range(8):
                at(-58 + i, memset_abct, i)

            # pending out-pairs: (gidx, c0, w0, w1)
            pend = []
            prev_w = [None] * ngroups
            evac_flip = [0]
            defer_out = []

            def flush_deferred_outs():
                half = n_iters * 256 // 2
                while defer_out:
                    gi2, osb = defer_out.pop(0)
                    b2, g2 = divmod(gi2, hg)
                    nc.gpsimd.dma_start(
                        out=out_d[b2, g2, :, half : 2 * half],
                        in_=osb[:, n_iters // 2 : n_iters, :],
                    )

            for w in range(1, nwaves):
                at(w * 64 + 8, flush_deferred_outs)

            lo = min(events)
            for s in range(lo, n_slots + 3):
                for fn, args in events.get(s, ()):
                    fn(*args)
                # interleave within the slot so no two consecutive matmuls
                # target the same PSUM bank, and DVE sees stt BEFORE evac:
                #   out(c0) [pout] .. seed [pg] .. out(c0+1) [pout]
                #   .. grad [pg] + stt .. evac
                po_info = None
                flush_ok = (s % slots_per_wave >= 8 or s >= n_slots
                            or len(pend) > 5)
                if pend and s >= 6 and flush_ok:
                    gq, c0q, w0q, w1q = pend.pop(0)
                    evac_flip[0] = (evac_flip[0] + 1) % 3
                    on_act = evac_flip[0] != 0  # 2/3 Act, 1/3 DVE
                    po = pout_pool.tile([128, 2, 256], f32, tag="po")
                    emit_out_mm(gq, c0q, w0q, po, 0)
                    po_info = (gq, c0q, w1q, po, on_act)
                in_chain = 0 <= s < n_slots
                if in_chain:
                    w, r = divmod(s, slots_per_wave)
                    c, gp = divmod(r, WAVE)
                    gidx = w * WAVE + gp
                    g = glist[gidx]
                    pg = chain_seed(g, c)
                if po_info is not None:
                    emit_out_mm(po_info[0], po_info[1] + 1, po_info[2],
                                po_info[3], 1)
                if in_chain:
                    chain_grad(g, c, pg)
                    if c % 2 == 1:
                        pend.append((gidx, c - 1, prev_w[gidx], g.wrep))
                    else:
                        prev_w[gidx] = g.wrep
                if po_info is not None:
                    emit_evac(po_info[0], po_info[1], po_info[3], po_info[4])
            while pend:
                gq, c0q, w0q, w1q = pend.pop(0)
                evac_flip[0] = (evac_flip[0] + 1) % 3
                po = pout_pool.tile([128, 2, 256], f32, tag="po")
                emit_out_mm(gq, c0q, w0q, po, 0)
                emit_out_mm(gq, c0q + 1, w1q, po, 1)
                emit_evac(gq, c0q, po, evac_flip[0] != 0)

    _split_excess_waits(nc)
    return nc


def _get_nc():
    if "nc" not in _CACHE:
        _CACHE["nc"] = _build()
    return _CACHE["nc"]


def _host_prep(q, k, v):
    """Host re-layout (token t = c*256 + j*128 + p)."""
    import ml_dtypes

    bf = ml_dtypes.bfloat16
    f8 = ml_dtypes.float8_e4m3
    Bq, Hq, Nq, Dq = q.shape
    hg = Hq // 2
    ni = Nq // 256
    # kv: [b, g, p, c, j, (k0|k1|v0|v1), d]
    k7 = k.reshape(Bq, hg, 2, ni, 2, 128, Dq)
    v7 = (-v).reshape(Bq, hg, 2, ni, 2, 128, Dq)
    kv = np.stack(
        [k7[:, :, 0], k7[:, :, 1], v7[:, :, 0], v7[:, :, 1]], axis=5
    )  # [b, g, c, j, p, 4, d]
    kv = np.ascontiguousarray(
        kv.transpose(0, 1, 4, 2, 3, 5, 6).reshape(Bq, hg, 128, ni * 2 * 4 * Dq)
    ).astype(f8)
    # qt: [b, g, hpair*64+d, c, t]
    q6 = q.reshape(Bq, hg, 2, ni, 256, Dq)
    qt = np.ascontiguousarray(
        q6.transpose(0, 1, 2, 5, 3, 4).reshape(Bq, hg, 128, ni * 256)
    ).astype(bf)
    return kv, qt


def _host_unshuffle(out_host):
    """[b, g, hpair*64+e, c*256+t] bf16 -> (B, N, H*64) f32."""
    Bq, hgq, _, w = out_host.shape
    ni = w // 256
    o6 = np.asarray(out_host, dtype=np.float32).reshape(
        Bq, hgq, 2, 64, ni, 256
    )
    # [b, g, hp, e, c, t] -> [b, c, t, g, hp, e]
    return np.ascontiguousarray(
        o6.transpose(0, 4, 5, 1, 2, 3).reshape(Bq, ni * 256, hgq * 2 * 64)
    )


def kernel(q, k, v, W_init, training=0, return_aux=0, **_unused):
    import ml_dtypes
    from concourse.bass_utils import run_bass_kernel_spmd

    q = np.asarray(q, dtype=np.float32)
    k = np.asarray(k, dtype=np.float32)
    v = np.asarray(v, dtype=np.float32)
    W_init = np.ascontiguousarray(np.asarray(W_init, dtype=np.float32))

    kv, qt = _host_prep(q, k, v)
    wbd = np.zeros((HG, 128, 128), dtype=np.float32)
    wbd[:, 0:64, 0:64] = W_init[0::2]
    wbd[:, 64:128, 64:128] = W_init[1::2]
    wbd = np.ascontiguousarray(
        wbd.transpose(1, 0, 2).reshape(128, HG * 128)
    )
    ident = np.eye(128, dtype=ml_dtypes.bfloat16)

    nc = _get_nc()
    in_maps = []
    for i in range(NCORES):
        sl = slice(i * NB, (i + 1) * NB)
        in_maps.append(
            {"qt": qt[sl], "kv": kv[sl], "W12bd": wbd, "ident": ident}
        )

    trace = bool(int(os.environ.get("BASS_KERNEL_TRACE", "0")))
    res = run_bass_kernel_spmd(
        nc, in_maps, core_ids=list(range(NCORES)), trace=trace
    )
    _CACHE["last_results"] = res
    out_host = np.concatenate(
        [np.asarray(res.results[i]["out"]) for i in range(NCORES)], axis=0
    )
    return _host_unshuffle(out_host)


if __name__ == "__main__":
    rng = np.random.default_rng(0)
    q = rng.standard_normal((B, H, N, D), dtype=np.float32)
    k = rng.standard_normal((B, H, N, D), dtype=np.float32)
    v = rng.standard_normal((B, H, N, D), dtype=np.float32)
    W = (rng.standard_normal((H, D, D)) * D**-0.5).astype(np.float32)
    out = kernel(q, k, v, W)
    print("kernel ran, out shape:", out.shape)


# revision 52
# speedup vs baseline: 1.2404x; 1.0612x over previous
"""Trainium2 Bass kernel for ARM TTT multi-head self-attention (inner-GD scan).

Math per (b, h) pair (B=16, H=12, N=4096, D=64, 16 chunks of m=256 tokens):
    A_i = k_i^T k_i ;  ct_i = k_i^T (-v_i)      (token contraction)
    grad_raw_i = A_i @ W_{i-1} + ct_i
    W_i = W_{i-1} - s * grad_raw_i,  s = 1/(m*D)
    out_i = q_i @ W_i
Pairs are fully independent -> shard B over the 8 NeuronCores (24 chains/core).

v6: v5's measured bottleneck was PE instruction CADENCE (~116ns per matmul
regardless of size: LdWeights + dispatch), 12 matmuls per chunk.  v6 packs
each head-PAIR into block-diagonal 128x128 operands -> 4 matmuls per chunk:

  1. act:  ONE fp8 DoubleRow matmul per chunk contracts all 256 tokens:
           lhsT = [k0|k1] (128t x 2j x 128), rhs = [k0|k1|v0|v1] (x 256)
           -> pac[128, 256]: A0/A1 diag blocks of cols 0:128, ct0/ct1 diag
              blocks of cols 128:256 (junk off-diag).  4 chunks per pac.
  2. cast: per t-block (4 chunks), per pair, ONE activation moves the A/ct
           diag blocks into PERSISTENT pre-zeroed block-diag bf16 tiles
           (abct) - zeros off the diag keep the chain closed in block-diag.
  3. seed: matmul(pg = Id^T @ ctbd)   [start of PSUM accumulation group]
  4. grad: matmul(pg += Abd^T @ Wbd)  [stop]
     stt (DVE, ONE op): Wbd' = -s*pg + Wbd   (off-diag stays 0: 0*s+0)
  5. out:  matmul(pout = Wbd'(lhsT) @ qt[128 dpair x 256 tok]) - both heads
           in one 256-col matmul; emitted in chunk-PAIRS, one evac per pair.

The serial W-chain round trip (PE->DVE->PE ~1us) is hidden by round-robining
chunks across a 4-group window; acts/casts lead by LAG slots; outs trail.
PSUM budget (8 banks): pac 2x2 + pg 2 + pout 2.
GpSimd CANNOT access PSUM on TRN2, so casts go to Act and evacs to DVE/Act.

Device layouts (token t = c*256 + j*128 + p):
    kv (per group):  [128(p), 16(c), 2(j), 4(k0|k1|v0|v1), 64]  fp8 (v negated)
    qt (per group):  [128(hpair*64+d), 16(c), 256(t=j*128+p)]   bf16
    out (per group): [128(hpair*64+e), 16(c), 256(t)]           bf16
    W12bd: [hg, 128, 128] f32 block-diag(W_h0, W_h1); carried chain is bf16.
"""

import os
import sys

sys.path.insert(0, "/opt/trn_rl_repo")

import numpy as np

B, H, N, D = 16, 12, 4096, 64
N_ITERS = 16
M = N // N_ITERS  # 256 tokens per chunk
NCORES = 8
NB = B // NCORES  # batches per core
HG = H // 2  # head-pair groups per batch
SCALE = 1.0 / (M * D)
WAVE = 4  # chain interleave width (groups round-robined per chunk)
LAG = 14  # slots the act/cast stream leads the chain stream
CB = 4  # chunks per t-block (pac granularity)
USE_DR = True  # fp8 DoubleRow: one act matmul per chunk (else 2, j-accum)
DEFER_OUT_DMA = False

_CACHE = {}


def _split_excess_waits(nc):
    """walrus in this env accepts at most ONE sem wait per instruction
    (two on EventSemaphore); move excess waits onto EventSemaphore
    instructions inserted just before on the same engine."""
    import concourse.mybir as mybir

    n_ev = 0
    for f in nc.m.functions:
        for b in f.blocks:
            il = b.instructions
            idx = 0
            while idx < len(il):
                inst = il[idx]
                si = getattr(inst, "sync_info", None)
                if si is not None and len(si.on_wait) > 1:
                    waits = list(si.on_wait)
                    si.on_wait = [waits[0]]
                    extra = waits[1:]
                    for g in range(0, len(extra), 2):
                        n_ev += 1
                        ev = mybir.InstEventSemaphore(
                            name=f"EVSPLIT-{n_ev}",
                            engine=inst.engine,
                            ins=[],
                            outs=[],
                            sync_info=mybir.SyncInfo(
                                on_wait=extra[g : g + 2], on_update=[]
                            ),
                        )
                        nc.register_instruction(ev)
                        il.insert(idx, ev)
                        idx += 1
                idx += 1
    return n_ev


class _G:
    __slots__ = ("kv", "qt", "outsb", "wrep", "abct", "pac", "b", "gi")


def _build(nb=NB, hg=HG, n_iters=N_ITERS):
    import concourse.bass as bass
    import concourse.mybir as mybir
    from concourse.tile import TileContext

    f32 = mybir.dt.float32
    bf16 = mybir.dt.bfloat16
    fp8 = mybir.dt.float8e4
    Copy = mybir.ActivationFunctionType.Copy
    mult = mybir.AluOpType.mult
    add = mybir.AluOpType.add
    DR = mybir.MatmulPerfMode.DoubleRow

    ngroups = nb * hg  # 12
    nwaves = ngroups // WAVE  # 3
    slots_per_wave = WAVE * n_iters  # 64
    n_tb = n_iters // CB  # 4 t-blocks per group

    nc = bass.Bass()
    q_d = nc.declare_dram_parameter(
        "qt", [nb, hg, 128, n_iters * 256], bf16, isOutput=False
    )
    kv_d = nc.declare_dram_parameter(
        "kv", [nb, hg, 128, n_iters * 2 * 4 * D], fp8, isOutput=False
    )
    w_d = nc.declare_dram_parameter("W12bd", [128, hg * 128], f32, isOutput=False)
    id_d = nc.declare_dram_parameter("ident", [128, 128], bf16, isOutput=False)
    out_d = nc.declare_dram_parameter(
        "out", [nb, hg, 128, n_iters * 256], bf16, isOutput=True
    )

    with TileContext(nc) as tc:
        with (
            tc.tile_pool(name="singles", bufs=1) as singles,
            tc.tile_pool(name="kv", bufs=8) as kv_pool,
            tc.tile_pool(name="qt", bufs=6) as qt_pool,
            tc.tile_pool(name="osb", bufs=6) as osb_pool,
            tc.tile_pool(name="abct", bufs=8) as abct_pool,
            tc.tile_pool(name="wrp", bufs=14) as wrp_pool,
            tc.tile_pool(name="pac", bufs=2, space="PSUM") as pac_pool,
            tc.tile_pool(name="pg", bufs=2, space="PSUM") as pg_pool,
            tc.tile_pool(name="pout", bufs=2, space="PSUM") as pout_pool,
        ):
            ident = singles.tile([128, 128], bf16)
            nc.sync.dma_start(out=ident, in_=id_d[:, :])
            # hoist the lazy ACT_TABLE_LOAD off the first real cast's path
            dummy = singles.tile([128, 1], bf16)
            nc.scalar.activation(
                dummy, ident[:, 0:1], func=Copy, scale=1.0
            )
            winit = singles.tile([128, hg, 128], f32)
            winit_dma = [False]

            def load_winit():
                winit_dma[0] = True
                nc.sync.dma_start(
                    out=winit, in_=w_d.rearrange("p (g e) -> p g e", g=hg)
                )

            # persistent abct rotation: casts only ever write the diag
            # blocks, so the one-time memset zeros persist across reuses
            # (same logical tensors, manual rotation).  The memsets are
            # emitted from the schedule (after wave-0 DMA triggers) so they
            # don't block the GpSimd DMA queue at startup.
            abct_tiles = []
            for _i in range(8):
                abt = abct_pool.tile([128, CB, 2, 128], bf16, tag="abct")
                abct_tiles.append(abt)
            abct_ctr = [0]

            def memset_abct(i):
                nc.gpsimd.memset(abct_tiles[i], 0.0)

            glist = [None] * ngroups

            def ensure_group(gidx):
                if glist[gidx] is not None:
                    return
                g = _G()
                g.b, g.gi = divmod(gidx, hg)
                g.wrep = None
                g.abct = {}
                g.pac = None
                g.kv = None
                g.qt = None
                g.outsb = None
                glist[gidx] = g

            def init_wrep(gidx):
                # emitted strictly after load_winit so the copy waits on it
                ensure_group(gidx)
                g = glist[gidx]
                g.wrep = wrp_pool.tile([128, 128], bf16, tag="wrep")
                nc.vector.tensor_copy(g.wrep, winit[:, g.gi, :])

            def _q(gidx):
                # single queue: transfers fair-share DMA engines, so issue
                # order IS the priority order - keep inputs on one queue
                return nc.sync

            def kv_part(gidx, h, nh):
                # kv DMA in 1/nh fractions (latency vs per-transfer overhead)
                ensure_group(gidx)
                g = glist[gidx]
                if g.kv is None:
                    g.kv = kv_pool.tile(
                        [128, n_iters, 2, 4, D], fp8, tag="kv"
                    )
                hc = n_iters // nh
                w2 = hc * 2 * 4 * D
                _q(gidx).dma_start(
                    out=g.kv[:, h * hc : (h + 1) * hc, :, :, :],
                    in_=kv_d[g.b, g.gi, :, h * w2 : (h + 1) * w2].rearrange(
                        "p (c j s d) -> p c j s d", j=2, s=4, d=D
                    ),
                )

            def qt_part(gidx, h, nh):
                g = glist[gidx]
                if g.qt is None:
                    g.qt = qt_pool.tile([128, n_iters, 256], bf16, tag="qt")
                hc = n_iters // nh
                w2 = hc * 256
                _q(gidx).dma_start(
                    out=g.qt[:, h * hc : (h + 1) * hc, :],
                    in_=q_d[g.b, g.gi, :, h * w2 : (h + 1) * w2].rearrange(
                        "p (c t) -> p c t", t=256
                    ),
                )

            def emit_act(gidx, tb, u):
                # chunk c = CB*tb + u Gram matmul into pac[:, u, :, :]
                g = glist[gidx]
                c = CB * tb + u
                if u == 0:
                    g.pac = pac_pool.tile([128, CB, 2, 128], f32, tag="pac")
                if USE_DR:
                    nc.tensor.matmul(
                        g.pac[:, u, :, :],
                        lhsT=g.kv[:, c, :, 0:2, :],
                        rhs=g.kv[:, c, :, :, :],
                        start=True, stop=True,
                        perf_mode=DR,
                        skip_group_check=True,
                    )
                else:
                    for j in (0, 1):
                        nc.tensor.matmul(
                            g.pac[:, u, :, :],
                            lhsT=g.kv[:, c, j, 0:2, :],
                            rhs=g.kv[:, c, j, :, :],
                            start=(j == 0), stop=(j == 1),
                            skip_group_check=True,
                        )

            def emit_cast(gidx, tb):
                # A/ct diag blocks -> block-diag bf16 (abct off-diag stays 0)
                g = glist[gidx]
                ab = abct_tiles[abct_ctr[0] % len(abct_tiles)]
                on_dve = abct_ctr[0] % 2 == 0
                abct_ctr[0] += 1
                nc.scalar.activation(
                    ab[0:64, :, :, 0:64], g.pac[0:64, :, :, 0:64],
                    func=Copy, scale=1.0,
                )
                if on_dve:
                    nc.vector.tensor_copy(
                        ab[64:128, :, :, 64:128], g.pac[64:128, :, :, 64:128]
                    )
                else:
                    nc.scalar.activation(
                        ab[64:128, :, :, 64:128], g.pac[64:128, :, :, 64:128],
                        func=Copy, scale=1.0,
                    )
                g.abct[tb] = ab
                g.pac = None

            def chain_seed(g, c):
                tb, u = divmod(c, CB)
                ab = g.abct[tb]
                pg = pg_pool.tile([128, 512], f32, tag="pg")
                nc.tensor.matmul(
                    pg[:, 0:128],
                    lhsT=ident[:, :],
                    rhs=ab[:, u, 1, :],
                    start=True, stop=False, skip_group_check=True,
                )
                return pg

            def chain_grad(g, c, pg):
                tb, u = divmod(c, CB)
                ab = g.abct[tb]
                nc.tensor.matmul(
                    pg[:, 0:128],
                    lhsT=ab[:, u, 0, :],
                    rhs=g.wrep[:, :],
                    start=False, stop=True, skip_group_check=True,
                )
                wnew = wrp_pool.tile([128, 128], bf16, tag="wrep")
                nc.vector.scalar_tensor_tensor(
                    wnew, pg[:, 0:128], -SCALE, g.wrep,
                    op0=mult, op1=add,
                )
                g.wrep = wnew
                if u == CB - 1:
                    del g.abct[tb]

            def emit_out_mm(gidx, c, wrep, po, slot_idx):
                g = glist[gidx]
                nc.tensor.matmul(
                    po[:, slot_idx, :], lhsT=wrep[:, :], rhs=g.qt[:, c, :],
                    start=True, stop=True, skip_group_check=True,
                )

            def emit_evac(gidx, c0, po, evac_on_act):
                g = glist[gidx]
                if g.outsb is None:
                    g.outsb = osb_pool.tile(
                        [128, n_iters, 256], bf16, tag="osb"
                    )
                dst = g.outsb[:, c0 : c0 + 2, :]
                if evac_on_act:
                    nc.scalar.activation(dst, po, func=Copy, scale=1.0)
                else:
                    nc.vector.tensor_copy(dst, po)
                half = n_iters * 256 // 2
                quart = half // 2
                last_wave = gidx >= ngroups - WAVE
                if last_wave:
                    # last wave: drain in quarters over the idle queues so
                    # the epilogue tail is short
                    oq = (nc.gpsimd, nc.sync, nc.scalar, nc.gpsimd)[gidx % 4]
                    if c0 + 1 == n_iters // 2 - 1:
                        oq.dma_start(
                            out=out_d[g.b, g.gi, :, 0:half],
                            in_=g.outsb[:, 0 : n_iters // 2, :],
                        )
                    elif c0 + 1 == 3 * n_iters // 4 - 1:
                        oq.dma_start(
                            out=out_d[g.b, g.gi, :, half : half + quart],
                            in_=g.outsb[:, n_iters // 2 : 3 * n_iters // 4, :],
                        )
                    elif c0 + 1 == n_iters - 1:
                        oq.dma_start(
                            out=out_d[g.b, g.gi, :, half + quart :],
                            in_=g.outsb[:, 3 * n_iters // 4 :, :],
                        )
                        g.outsb = None
                        g.qt = None
                        g.kv = None
                    return
                if c0 + 1 == n_iters // 2 - 1:
                    nc.gpsimd.dma_start(
                        out=out_d[g.b, g.gi, :, 0:half],
                        in_=g.outsb[:, 0 : n_iters // 2, :],
                    )
                elif c0 + 1 == n_iters - 1:
                    if DEFER_OUT_DMA:
                        # defer the 2nd-half DMA past the wave boundary so
                        # it doesn't steal HBM from the next wave's kv/qt
                        defer_out.append((gidx, g.outsb))
                    else:
                        nc.gpsimd.dma_start(
                            out=out_d[g.b, g.gi, :, half : 2 * half],
                            in_=g.outsb[:, n_iters // 2 : n_iters, :],
                        )
                    g.outsb = None
                    g.qt = None
                    g.kv = None

            # ---------------- schedule -----------------------------------
            # chain slot s (0..191): wave w = s//64, r = s%64, c = r//WAVE,
            #   gp = r%WAVE, group g = w*WAVE+gp.
            # act item (g, tb): 4 DR matmuls at slots w*64+16*tb+gp-LAG ...
            #   +3, cast at +4.
            # group kv DMA one wave ahead (spread), qt half a wave ahead.
            events = {}

            def at(slot, fn, *args):
                events.setdefault(slot, []).append((fn, args))

            n_slots = nwaves * slots_per_wave
            for w in range(nwaves):
                for gp in range(WAVE):
                    gidx = w * WAVE + gp
                    if w == 0:
                        # wave 0: kv quarters, just-in-time priority order
                        # (first chain slots gate on the least DMA bytes)
                        at(-50 + gp, kv_part, gidx, 0, 4)
                        at(-45 + gp, kv_part, gidx, 1, 4)
                        at(-40 + gp, qt_part, gidx, 0, 2)
                        at(-36 + gp, kv_part, gidx, 2, 4)
                        at(-32 + gp, qt_part, gidx, 1, 2)
                        at(-28 + gp, kv_part, gidx, 3, 4)
                    else:
                        # kv one wave early (highest priority after wave 0);
                        # qt later - first needed only ~6 slots into a wave
                        at(w * 64 - 88 + 4 * gp, kv_part, gidx, 0, 1)
                        at(w * 64 - 71 + 4 * gp, qt_part, gidx, 0, 1)
                    at(w * 64 - 43 + gp if w == 0 else w * 64 - 86 + 4 * gp,
                       init_wrep, gidx)
                    for tb in range(n_tb):
                        t0 = w * 64 + 16 * tb + 4 * gp - LAG
                        for u in range(CB):
                            at(t0 + u, emit_act, gidx, tb, u)
                        at(t0 + CB, emit_cast, gidx, tb)
            at(-46, load_winit)
            for i in range(8):
                at(-58 + i, memset_abct, i)

            # pending out-pairs: (gidx, c0, w0, w1)
            pend = []
            prev_w = [None] * ngroups
            evac_flip = [0]
            defer_out = []

            def flush_deferred_outs():
                half = n_iters * 256 // 2
                while defer_out:
                    gi2, osb = defer_out.pop(0)
                    b2, g2 = divmod(gi2, hg)
                    nc.gpsimd.dma_start(
                        out=out_d[b2, g2, :, half : 2 * half],
                        in_=osb[:, n_iters // 2 : n_iters, :],
                    )

            for w in range(1, nwaves):
                at(w * 64 + 8, flush_deferred_outs)

            lo = min(events)
            for s in range(lo, n_slots + 3):
                # slot order: chain-critical ops first (outs, seed, grad,
                # stt, evac), then act/cast/DMA events - so the engine
                # queues see latency-critical work ahead of slack work,
                # and no two consecutive matmuls share a PSUM bank.
                po_info = None
                flush_ok = (s % slots_per_wave >= 8 or s >= n_slots
                            or len(pend) > 5)
                if pend and s >= 6 and flush_ok:
                    gq, c0q, w0q, w1q = pend.pop(0)
                    evac_flip[0] = (evac_flip[0] + 1) % 3
                    on_act = evac_flip[0] != 0  # 2/3 Act, 1/3 DVE
                    po = pout_pool.tile([128, 2, 256], f32, tag="po")
                    emit_out_mm(gq, c0q, w0q, po, 0)
                    po_info = (gq, c0q, w1q, po, on_act)
                in_chain = 0 <= s < n_slots
                if in_chain:
                    w, r = divmod(s, slots_per_wave)
                    c, gp = divmod(r, WAVE)
                    gidx = w * WAVE + gp
                    g = glist[gidx]
                    pg = chain_seed(g, c)
                if po_info is not None:
                    emit_out_mm(po_info[0], po_info[1] + 1, po_info[2],
                                po_info[3], 1)
                if in_chain:
                    chain_grad(g, c, pg)
                    if c % 2 == 1:
                        pend.append((gidx, c - 1, prev_w[gidx], g.wrep))
                    else:
                        prev_w[gidx] = g.wrep
                if po_info is not None:
                    emit_evac(po_info[0], po_info[1], po_info[3], po_info[4])
                for fn, args in events.get(s, ()):
                    fn(*args)
            while pend:
                gq, c0q, w0q, w1q = pend.pop(0)
                evac_flip[0] = (evac_flip[0] + 1) % 3
                po = pout_pool.tile([128, 2, 256], f32, tag="po")
                emit_out_mm(gq, c0q, w0q, po, 0)
                emit_out_mm(gq, c0q + 1, w1q, po, 1)
                emit_evac(gq, c0q, po, evac_flip[0] != 0)

    _split_excess_waits(nc)
    return nc


def _get_nc():
    if "nc" not in _CACHE:
        _CACHE["nc"] = _build()
    return _CACHE["nc"]


def _host_prep(q, k, v):
    """Host re-layout (token t = c*256 + j*128 + p)."""
    import ml_dtypes

    bf = ml_dtypes.bfloat16
    f8 = ml_dtypes.float8_e4m3
    Bq, Hq, Nq, Dq = q.shape
    hg = Hq // 2
    ni = Nq // 256
    # kv: [b, g, p, c, j, (k0|k1|v0|v1), d]
    k7 = k.reshape(Bq, hg, 2, ni, 2, 128, Dq)
    v7 = (-v).reshape(Bq, hg, 2, ni, 2, 128, Dq)
    kv = np.stack(
        [k7[:, :, 0], k7[:, :, 1], v7[:, :, 0], v7[:, :, 1]], axis=5
    )  # [b, g, c, j, p, 4, d]
    kv = np.ascontiguousarray(
        kv.transpose(0, 1, 4, 2, 3, 5, 6).reshape(Bq, hg, 128, ni * 2 * 4 * Dq)
    ).astype(f8)
    # qt: [b, g, hpair*64+d, c, t]
    q6 = q.reshape(Bq, hg, 2, ni, 256, Dq)
    qt = np.ascontiguousarray(
        q6.transpose(0, 1, 2, 5, 3, 4).reshape(Bq, hg, 128, ni * 256)
    ).astype(bf)
    return kv, qt


def _host_unshuffle(out_host):
    """[b, g, hpair*64+e, c*256+t] bf16 -> (B, N, H*64) f32."""
    Bq, hgq, _, w = out_host.shape
    ni = w // 256
    o6 = np.asarray(out_host, dtype=np.float32).reshape(
        Bq, hgq, 2, 64, ni, 256
    )
    # [b, g, hp, e, c, t] -> [b, c, t, g, hp, e]
    return np.ascontiguousarray(
        o6.transpose(0, 4, 5, 1, 2, 3).reshape(Bq, ni * 256, hgq * 2 * 64)
    )


def kernel(q, k, v, W_init, training=0, return_aux=0, **_unused):
    import ml_dtypes
    from concourse.bass_utils import run_bass_kernel_spmd

    q = np.asarray(q, dtype=np.float32)
    k = np.asarray(k, dtype=np.float32)
    v = np.asarray(v, dtype=np.float32)
    W_init = np.ascontiguousarray(np.asarray(W_init, dtype=np.float32))

    kv, qt = _host_prep(q, k, v)
    wbd = np.zeros((HG, 128, 128), dtype=np.float32)
    wbd[:, 0:64, 0:64] = W_init[0::2]
    wbd[:, 64:128, 64:128] = W_init[1::2]
    wbd = np.ascontiguousarray(
        wbd.transpose(1, 0, 2).reshape(128, HG * 128)
    )
    ident = np.eye(128, dtype=ml_dtypes.bfloat16)

    nc = _get_nc()
    in_maps = []
    for i in range(NCORES):
        sl = slice(i * NB, (i + 1) * NB)
        in_maps.append(
            {"qt": qt[sl], "kv": kv[sl], "W12bd": wbd, "ident": ident}
        )

    trace = bool(int(os.environ.get("BASS_KERNEL_TRACE", "0")))
    res = run_bass_kernel_spmd(
        nc, in_maps, core_ids=list(range(NCORES)), trace=trace
    )
    _CACHE["last_results"] = res
    out_host = np.concatenate(
        [np.asarray(res.results[i]["out"]) for i in range(NCORES)], axis=0
    )
    return _host_unshuffle(out_host)


if __name__ == "__main__":
    rng = np.random.default_rng(0)
    q = rng.standard_normal((B, H, N, D), dtype=np.float32)
    k = rng.standard_normal((B, H, N, D), dtype=np.float32)
    v = rng.standard_normal((B, H, N, D), dtype=np.float32)
    W = (rng.standard_normal((H, D, D)) * D**-0.5).astype(np.float32)
    out = kernel(q, k, v, W)
    print("kernel ran, out shape:", out.shape)


# revision 53
# speedup vs baseline: 1.2469x; 1.0052x over previous
"""Trainium2 Bass kernel for ARM TTT multi-head self-attention (inner-GD scan).

Math per (b, h) pair (B=16, H=12, N=4096, D=64, 16 chunks of m=256 tokens):
    A_i = k_i^T k_i ;  ct_i = k_i^T (-v_i)      (token contraction)
    grad_raw_i = A_i @ W_{i-1} + ct_i
    W_i = W_{i-1} - s * grad_raw_i,  s = 1/(m*D)
    out_i = q_i @ W_i
Pairs are fully independent -> shard B over the 8 NeuronCores (24 chains/core).

v6: v5's measured bottleneck was PE instruction CADENCE (~116ns per matmul
regardless of size: LdWeights + dispatch), 12 matmuls per chunk.  v6 packs
each head-PAIR into block-diagonal 128x128 operands -> 4 matmuls per chunk:

  1. act:  ONE fp8 DoubleRow matmul per chunk contracts all 256 tokens:
           lhsT = [k0|k1] (128t x 2j x 128), rhs = [k0|k1|v0|v1] (x 256)
           -> pac[128, 256]: A0/A1 diag blocks of cols 0:128, ct0/ct1 diag
              blocks of cols 128:256 (junk off-diag).  4 chunks per pac.
  2. cast: per t-block (4 chunks), per pair, ONE activation moves the A/ct
           diag blocks into PERSISTENT pre-zeroed block-diag bf16 tiles
           (abct) - zeros off the diag keep the chain closed in block-diag.
  3. seed: matmul(pg = Id^T @ ctbd)   [start of PSUM accumulation group]
  4. grad: matmul(pg += Abd^T @ Wbd)  [stop]
     stt (DVE, ONE op): Wbd' = -s*pg + Wbd   (off-diag stays 0: 0*s+0)
  5. out:  matmul(pout = Wbd'(lhsT) @ qt[128 dpair x 256 tok]) - both heads
           in one 256-col matmul; emitted in chunk-PAIRS, one evac per pair.

The serial W-chain round trip (PE->DVE->PE ~1us) is hidden by round-robining
chunks across a 4-group window; acts/casts lead by LAG slots; outs trail.
PSUM budget (8 banks): pac 2x2 + pg 2 + pout 2.
GpSimd CANNOT access PSUM on TRN2, so casts go to Act and evacs to DVE/Act.

Device layouts (token t = c*256 + j*128 + p):
    kv (per group):  [128(p), 16(c), 2(j), 4(k0|k1|v0|v1), 64]  fp8 (v negated)
    qt (per group):  [128(hpair*64+d), 16(c), 256(t=j*128+p)]   bf16
    out (per group): [128(hpair*64+e), 16(c), 256(t)]           bf16
    W12bd: [hg, 128, 128] f32 block-diag(W_h0, W_h1); carried chain is bf16.
"""

import os
import sys

sys.path.insert(0, "/opt/trn_rl_repo")

import numpy as np

B, H, N, D = 16, 12, 4096, 64
N_ITERS = 16
M = N // N_ITERS  # 256 tokens per chunk
NCORES = 8
NB = B // NCORES  # batches per core
HG = H // 2  # head-pair groups per batch
SCALE = 1.0 / (M * D)
WAVE = 4  # chain interleave width (groups round-robined per chunk)
LAG = 14  # slots the act/cast stream leads the chain stream
CB = 4  # chunks per t-block (pac granularity)
USE_DR = True  # fp8 DoubleRow: one act matmul per chunk (else 2, j-accum)
DEFER_OUT_DMA = False

_CACHE = {}


def _split_excess_waits(nc):
    """walrus in this env accepts at most ONE sem wait per instruction
    (two on EventSemaphore); move excess waits onto EventSemaphore
    instructions inserted just before on the same engine."""
    import concourse.mybir as mybir

    n_ev = 0
    for f in nc.m.functions:
        for b in f.blocks:
            il = b.instructions
            idx = 0
            while idx < len(il):
                inst = il[idx]
                si = getattr(inst, "sync_info", None)
                if si is not None and len(si.on_wait) > 1:
                    waits = list(si.on_wait)
                    si.on_wait = [waits[0]]
                    extra = waits[1:]
                    for g in range(0, len(extra), 2):
                        n_ev += 1
                        ev = mybir.InstEventSemaphore(
                            name=f"EVSPLIT-{n_ev}",
                            engine=inst.engine,
                            ins=[],
                            outs=[],
                            sync_info=mybir.SyncInfo(
                                on_wait=extra[g : g + 2], on_update=[]
                            ),
                        )
                        nc.register_instruction(ev)
                        il.insert(idx, ev)
                        idx += 1
                idx += 1
    return n_ev


class _G:
    __slots__ = ("kv", "qt", "outsb", "wrep", "abct", "pac", "b", "gi")


def _build(nb=NB, hg=HG, n_iters=N_ITERS):
    import concourse.bass as bass
    import concourse.mybir as mybir
    from concourse.tile import TileContext

    f32 = mybir.dt.float32
    bf16 = mybir.dt.bfloat16
    fp8 = mybir.dt.float8e4
    Copy = mybir.ActivationFunctionType.Copy
    mult = mybir.AluOpType.mult
    add = mybir.AluOpType.add
    DR = mybir.MatmulPerfMode.DoubleRow

    ngroups = nb * hg  # 12
    nwaves = ngroups // WAVE  # 3
    slots_per_wave = WAVE * n_iters  # 64
    n_tb = n_iters // CB  # 4 t-blocks per group

    nc = bass.Bass()
    q_d = nc.declare_dram_parameter(
        "qt", [nb, hg, 128, n_iters * 256], bf16, isOutput=False
    )
    kv_d = nc.declare_dram_parameter(
        "kv", [nb, hg, 128, n_iters * 2 * 4 * D], fp8, isOutput=False
    )
    w_d = nc.declare_dram_parameter("W12bd", [128, hg * 128], f32, isOutput=False)
    id_d = nc.declare_dram_parameter("ident", [128, 128], bf16, isOutput=False)
    out_d = nc.declare_dram_parameter(
        "out", [nb, hg, 128, n_iters * 256], bf16, isOutput=True
    )

    with TileContext(nc) as tc:
        with (
            tc.tile_pool(name="singles", bufs=1) as singles,
            tc.tile_pool(name="kv", bufs=8) as kv_pool,
            tc.tile_pool(name="qt", bufs=6) as qt_pool,
            tc.tile_pool(name="osb", bufs=6) as osb_pool,
            tc.tile_pool(name="abct", bufs=8) as abct_pool,
            tc.tile_pool(name="wrp", bufs=14) as wrp_pool,
            tc.tile_pool(name="pac", bufs=2, space="PSUM") as pac_pool,
            tc.tile_pool(name="pg", bufs=2, space="PSUM") as pg_pool,
            tc.tile_pool(name="pout", bufs=2, space="PSUM") as pout_pool,
        ):
            ident = singles.tile([128, 128], bf16)
            nc.sync.dma_start(out=ident, in_=id_d[:, :])
            # hoist the lazy ACT_TABLE_LOAD off the first real cast's path
            dummy = singles.tile([128, 1], bf16)
            nc.scalar.activation(
                dummy, ident[:, 0:1], func=Copy, scale=1.0
            )
            winit = singles.tile([128, hg, 128], f32)
            winit_dma = [False]

            def load_winit():
                winit_dma[0] = True
                nc.sync.dma_start(
                    out=winit, in_=w_d.rearrange("p (g e) -> p g e", g=hg)
                )

            # persistent abct rotation: casts only ever write the diag
            # blocks, so the one-time memset zeros persist across reuses
            # (same logical tensors, manual rotation).  The memsets are
            # emitted from the schedule (after wave-0 DMA triggers) so they
            # don't block the GpSimd DMA queue at startup.
            abct_tiles = []
            for _i in range(8):
                abt = abct_pool.tile([128, CB, 2, 128], bf16, tag="abct")
                abct_tiles.append(abt)
            abct_ctr = [0]

            def memset_abct(i):
                nc.gpsimd.memset(abct_tiles[i], 0.0)

            glist = [None] * ngroups

            def ensure_group(gidx):
                if glist[gidx] is not None:
                    return
                g = _G()
                g.b, g.gi = divmod(gidx, hg)
                g.wrep = None
                g.abct = {}
                g.pac = None
                g.kv = None
                g.qt = None
                g.outsb = None
                glist[gidx] = g

            def init_wrep(gidx):
                # emitted strictly after load_winit so the copy waits on it
                ensure_group(gidx)
                g = glist[gidx]
                g.wrep = wrp_pool.tile([128, 128], bf16, tag="wrep")
                nc.vector.tensor_copy(g.wrep, winit[:, g.gi, :])

            def _q(gidx):
                # single queue: transfers fair-share DMA engines, so issue
                # order IS the priority order - keep inputs on one queue
                return nc.sync

            def kv_part(gidx, h, nh):
                # kv DMA in 1/nh fractions (latency vs per-transfer overhead)
                ensure_group(gidx)
                g = glist[gidx]
                if g.kv is None:
                    g.kv = kv_pool.tile(
                        [128, n_iters, 2, 4, D], fp8, tag="kv"
                    )
                hc = n_iters // nh
                w2 = hc * 2 * 4 * D
                _q(gidx).dma_start(
                    out=g.kv[:, h * hc : (h + 1) * hc, :, :, :],
                    in_=kv_d[g.b, g.gi, :, h * w2 : (h + 1) * w2].rearrange(
                        "p (c j s d) -> p c j s d", j=2, s=4, d=D
                    ),
                )

            def qt_part(gidx, h, nh):
                g = glist[gidx]
                if g.qt is None:
                    g.qt = qt_pool.tile([128, n_iters, 256], bf16, tag="qt")
                hc = n_iters // nh
                w2 = hc * 256
                _q(gidx).dma_start(
                    out=g.qt[:, h * hc : (h + 1) * hc, :],
                    in_=q_d[g.b, g.gi, :, h * w2 : (h + 1) * w2].rearrange(
                        "p (c t) -> p c t", t=256
                    ),
                )

            def emit_act(gidx, tb, u):
                # chunk c = CB*tb + u Gram matmul into pac[:, u, :, :]
                g = glist[gidx]
                c = CB * tb + u
                if u == 0:
                    g.pac = pac_pool.tile([128, CB, 2, 128], f32, tag="pac")
                if USE_DR:
                    nc.tensor.matmul(
                        g.pac[:, u, :, :],
                        lhsT=g.kv[:, c, :, 0:2, :],
                        rhs=g.kv[:, c, :, :, :],
                        start=True, stop=True,
                        perf_mode=DR,
                        skip_group_check=True,
                    )
                else:
                    for j in (0, 1):
                        nc.tensor.matmul(
                            g.pac[:, u, :, :],
                            lhsT=g.kv[:, c, j, 0:2, :],
                            rhs=g.kv[:, c, j, :, :],
                            start=(j == 0), stop=(j == 1),
                            skip_group_check=True,
                        )

            def emit_cast(gidx, tb):
                # A/ct diag blocks -> block-diag bf16 (abct off-diag stays 0)
                g = glist[gidx]
                ab = abct_tiles[abct_ctr[0] % len(abct_tiles)]
                on_dve = abct_ctr[0] % 2 == 0
                abct_ctr[0] += 1
                nc.scalar.activation(
                    ab[0:64, :, :, 0:64], g.pac[0:64, :, :, 0:64],
                    func=Copy, scale=1.0,
                )
                if on_dve:
                    nc.vector.tensor_copy(
                        ab[64:128, :, :, 64:128], g.pac[64:128, :, :, 64:128]
                    )
                else:
                    nc.scalar.activation(
                        ab[64:128, :, :, 64:128], g.pac[64:128, :, :, 64:128],
                        func=Copy, scale=1.0,
                    )
                g.abct[tb] = ab
                g.pac = None

            def chain_seed(g, c):
                tb, u = divmod(c, CB)
                ab = g.abct[tb]
                pg = pg_pool.tile([128, 512], f32, tag="pg")
                nc.tensor.matmul(
                    pg[:, 0:128],
                    lhsT=ident[:, :],
                    rhs=ab[:, u, 1, :],
                    start=True, stop=False, skip_group_check=True,
                )
                return pg

            def chain_grad(g, c, pg):
                tb, u = divmod(c, CB)
                ab = g.abct[tb]
                nc.tensor.matmul(
                    pg[:, 0:128],
                    lhsT=ab[:, u, 0, :],
                    rhs=g.wrep[:, :],
                    start=False, stop=True, skip_group_check=True,
                )
                wnew = wrp_pool.tile([128, 128], bf16, tag="wrep")
                nc.vector.scalar_tensor_tensor(
                    wnew, pg[:, 0:128], -SCALE, g.wrep,
                    op0=mult, op1=add,
                )
                g.wrep = wnew
                if u == CB - 1:
                    del g.abct[tb]

            def emit_out_mm(gidx, c, wrep, po, slot_idx):
                g = glist[gidx]
                nc.tensor.matmul(
                    po[:, slot_idx, :], lhsT=wrep[:, :], rhs=g.qt[:, c, :],
                    start=True, stop=True, skip_group_check=True,
                )

            def emit_evac(gidx, c0, po, evac_on_act):
                g = glist[gidx]
                if g.outsb is None:
                    g.outsb = osb_pool.tile(
                        [128, n_iters, 256], bf16, tag="osb"
                    )
                dst = g.outsb[:, c0 : c0 + 2, :]
                if evac_on_act:
                    nc.scalar.activation(dst, po, func=Copy, scale=1.0)
                else:
                    nc.vector.tensor_copy(dst, po)
                half = n_iters * 256 // 2
                quart = half // 2
                last_wave = gidx >= ngroups - WAVE
                if last_wave:
                    # last wave: drain in quarters over the idle queues so
                    # the epilogue tail is short
                    oq = (nc.gpsimd, nc.sync, nc.scalar, nc.gpsimd)[gidx % 4]
                    if c0 + 1 == n_iters // 2 - 1:
                        oq.dma_start(
                            out=out_d[g.b, g.gi, :, 0:half],
                            in_=g.outsb[:, 0 : n_iters // 2, :],
                        )
                    elif c0 + 1 == 3 * n_iters // 4 - 1:
                        oq.dma_start(
                            out=out_d[g.b, g.gi, :, half : half + quart],
                            in_=g.outsb[:, n_iters // 2 : 3 * n_iters // 4, :],
                        )
                    elif c0 + 1 == n_iters - 1:
                        oq.dma_start(
                            out=out_d[g.b, g.gi, :, half + quart :],
                            in_=g.outsb[:, 3 * n_iters // 4 :, :],
                        )
                        g.outsb = None
                        g.qt = None
                        g.kv = None
                    return
                if c0 + 1 == n_iters // 2 - 1:
                    nc.gpsimd.dma_start(
                        out=out_d[g.b, g.gi, :, 0:half],
                        in_=g.outsb[:, 0 : n_iters // 2, :],
                    )
                elif c0 + 1 == n_iters - 1:
                    if DEFER_OUT_DMA:
                        # defer the 2nd-half DMA past the wave boundary so
                        # it doesn't steal HBM from the next wave's kv/qt
                        defer_out.append((gidx, g.outsb))
                    else:
                        nc.gpsimd.dma_start(
                            out=out_d[g.b, g.gi, :, half : 2 * half],
                            in_=g.outsb[:, n_iters // 2 : n_iters, :],
                        )
                    g.outsb = None
                    g.qt = None
                    g.kv = None

            # ---------------- schedule -----------------------------------
            # chain slot s (0..191): wave w = s//64, r = s%64, c = r//WAVE,
            #   gp = r%WAVE, group g = w*WAVE+gp.
            # act item (g, tb): 4 DR matmuls at slots w*64+16*tb+gp-LAG ...
            #   +3, cast at +4.
            # group kv DMA one wave ahead (spread), qt half a wave ahead.
            events = {}

            def at(slot, fn, *args):
                events.setdefault(slot, []).append((fn, args))

            n_slots = nwaves * slots_per_wave
            for w in range(nwaves):
                for gp in range(WAVE):
                    gidx = w * WAVE + gp
                    if w == 0:
                        # wave 0: kv quarters, just-in-time priority order
                        # (first chain slots gate on the least DMA bytes)
                        at(-50 + gp, kv_part, gidx, 0, 4)
                        at(-45 + gp, kv_part, gidx, 1, 4)
                        at(-40 + gp, qt_part, gidx, 0, 2)
                        at(-36 + gp, kv_part, gidx, 2, 4)
                        at(-32 + gp, qt_part, gidx, 1, 2)
                        at(-28 + gp, kv_part, gidx, 3, 4)
                    else:
                        # kv one wave early (highest priority after wave 0);
                        # qt later - first needed only ~6 slots into a wave
                        at(w * 64 - 88 + 4 * gp, kv_part, gidx, 0, 1)
                        at(w * 64 - 71 + 4 * gp, qt_part, gidx, 0, 1)
                    at(w * 64 - 43 + gp if w == 0 else w * 64 - 86 + 4 * gp,
                       init_wrep, gidx)
                    for tb in range(n_tb):
                        t0 = w * 64 + 16 * tb + 4 * gp - LAG
                        for u in range(CB):
                            at(t0 + u, emit_act, gidx, tb, u)
                        at(t0 + CB, emit_cast, gidx, tb)
            at(-46, load_winit)
            for i in range(8):
                at(-58 + i, memset_abct, i)

            # pending out-pairs: (gidx, c0, w0, w1)
            pend = []
            prev_w = [None] * ngroups
            evac_flip = [0]
            defer_out = []

            def flush_deferred_outs():
                half = n_iters * 256 // 2
                while defer_out:
                    gi2, osb = defer_out.pop(0)
                    b2, g2 = divmod(gi2, hg)
                    nc.gpsimd.dma_start(
                        out=out_d[b2, g2, :, half : 2 * half],
                        in_=osb[:, n_iters // 2 : n_iters, :],
                    )

            for w in range(1, nwaves):
                at(w * 64 + 8, flush_deferred_outs)

            lo = min(events)
            for s in range(lo, n_slots + 3):
                # slot order: chain-critical ops first (outs, seed, grad,
                # stt, evac), then act/cast/DMA events - so the engine
                # queues see latency-critical work ahead of slack work,
                # and no two consecutive matmuls share a PSUM bank.
                po_info = None
                flush_ok = (s % slots_per_wave >= 8 or s >= n_slots
                            or len(pend) > 5)
                if pend and s >= 6 and flush_ok:
                    gq, c0q, w0q, w1q = pend.pop(0)
                    evac_flip[0] = (evac_flip[0] + 1) % 3
                    on_act = evac_flip[0] != 0  # 2/3 Act, 1/3 DVE
                    po = pout_pool.tile([128, 2, 256], f32, tag="po")
                    emit_out_mm(gq, c0q, w0q, po, 0)
                    po_info = (gq, c0q, w1q, po, on_act)
                in_chain = 0 <= s < n_slots
                if in_chain:
                    w, r = divmod(s, slots_per_wave)
                    c, gp = divmod(r, WAVE)
                    gidx = w * WAVE + gp
                    g = glist[gidx]
                    pg = chain_seed(g, c)
                    chain_grad(g, c, pg)
                    if c % 2 == 1:
                        pend.append((gidx, c - 1, prev_w[gidx], g.wrep))
                    else:
                        prev_w[gidx] = g.wrep
                if po_info is not None:
                    emit_out_mm(po_info[0], po_info[1] + 1, po_info[2],
                                po_info[3], 1)
                if po_info is not None:
                    emit_evac(po_info[0], po_info[1], po_info[3], po_info[4])
                for fn, args in events.get(s, ()):
                    fn(*args)
            while pend:
                gq, c0q, w0q, w1q = pend.pop(0)
                evac_flip[0] = (evac_flip[0] + 1) % 3
                po = pout_pool.tile([128, 2, 256], f32, tag="po")
                emit_out_mm(gq, c0q, w0q, po, 0)
                emit_out_mm(gq, c0q + 1, w1q, po, 1)
                emit_evac(gq, c0q, po, evac_flip[0] != 0)

    _split_excess_waits(nc)
    return nc


def _get_nc():
    if "nc" not in _CACHE:
        _CACHE["nc"] = _build()
    return _CACHE["nc"]


def _host_prep(q, k, v):
    """Host re-layout (token t = c*256 + j*128 + p)."""
    import ml_dtypes

    bf = ml_dtypes.bfloat16
    f8 = ml_dtypes.float8_e4m3
    Bq, Hq, Nq, Dq = q.shape
    hg = Hq // 2
    ni = Nq // 256
    # kv: [b, g, p, c, j, (k0|k1|v0|v1), d]
    k7 = k.reshape(Bq, hg, 2, ni, 2, 128, Dq)
    v7 = (-v).reshape(Bq, hg, 2, ni, 2, 128, Dq)
    kv = np.stack(
        [k7[:, :, 0], k7[:, :, 1], v7[:, :, 0], v7[:, :, 1]], axis=5
    )  # [b, g, c, j, p, 4, d]
    kv = np.ascontiguousarray(
        kv.transpose(0, 1, 4, 2, 3, 5, 6).reshape(Bq, hg, 128, ni * 2 * 4 * Dq)
    ).astype(f8)
    # qt: [b, g, hpair*64+d, c, t]
    q6 = q.reshape(Bq, hg, 2, ni, 256, Dq)
    qt = np.ascontiguousarray(
        q6.transpose(0, 1, 2, 5, 3, 4).reshape(Bq, hg, 128, ni * 256)
    ).astype(bf)
    return kv, qt


def _host_unshuffle(out_host):
    """[b, g, hpair*64+e, c*256+t] bf16 -> (B, N, H*64) f32."""
    Bq, hgq, _, w = out_host.shape
    ni = w // 256
    o6 = np.asarray(out_host, dtype=np.float32).reshape(
        Bq, hgq, 2, 64, ni, 256
    )
    # [b, g, hp, e, c, t] -> [b, c, t, g, hp, e]
    return np.ascontiguousarray(
        o6.transpose(0, 4, 5, 1, 2, 3).reshape(Bq, ni * 256, hgq * 2 * 64)
    )


def kernel(q, k, v, W_init, training=0, return_aux=0, **_unused):
    import ml_dtypes
    from concourse.bass_utils import run_bass_kernel_spmd

    q = np.asarray(q, dtype=np.float32)
    k = np.asarray(k, dtype=np.float32)
    v = np.asarray(v, dtype=np.float32)
    W_init = np.ascontiguousarray(np.asarray(W_init, dtype=np.float32))

    kv, qt = _host_prep(q, k, v)
    wbd = np.zeros((HG, 128, 128), dtype=np.float32)
    wbd[:, 0:64, 0:64] = W_init[0::2]
    wbd[:, 64:128, 64:128] = W_init[1::2]
    wbd = np.ascontiguousarray(
        wbd.transpose(1, 0, 2).reshape(128, HG * 128)
    )
    ident = np.eye(128, dtype=ml_dtypes.bfloat16)

    nc = _get_nc()
    in_maps = []
    for i in range(NCORES):
        sl = slice(i * NB, (i + 1) * NB)
        in_maps.append(
            {"qt": qt[sl], "kv": kv[sl], "W12bd": wbd, "ident": ident}
        )

    trace = bool(int(os.environ.get("BASS_KERNEL_TRACE", "0")))
    res = run_bass_kernel_spmd(
        nc, in_maps, core_ids=list(range(NCORES)), trace=trace
    )
    _CACHE["last_results"] = res
    out_host = np.concatenate(
        [np.asarray(res.results[i]["out"]) for i in range(NCORES)], axis=0
    )
    return _host_unshuffle(out_host)


if __name__ == "__main__":
    rng = np.random.default_rng(0)
    q = rng.standard_normal((B, H, N, D), dtype=np.float32)
    k = rng.standard_normal((B, H, N, D), dtype=np.float32)
    v = rng.standard_normal((B, H, N, D), dtype=np.float32)
    W = (rng.standard_normal((H, D, D)) * D**-0.5).astype(np.float32)
    out = kernel(q, k, v, W)
    print("kernel ran, out shape:", out.shape)


# revision 60
# speedup vs baseline: 1.2576x; 1.0086x over previous
"""Trainium2 Bass kernel for ARM TTT multi-head self-attention (inner-GD scan).

Math per (b, h) pair (B=16, H=12, N=4096, D=64, 16 chunks of m=256 tokens):
    A_i = k_i^T k_i ;  ct_i = k_i^T (-v_i)      (token contraction)
    grad_raw_i = A_i @ W_{i-1} + ct_i
    W_i = W_{i-1} - s * grad_raw_i,  s = 1/(m*D)
    out_i = q_i @ W_i
Pairs are fully independent -> shard B over the 8 NeuronCores (24 chains/core).

v6: v5's measured bottleneck was PE instruction CADENCE (~116ns per matmul
regardless of size: LdWeights + dispatch), 12 matmuls per chunk.  v6 packs
each head-PAIR into block-diagonal 128x128 operands -> 4 matmuls per chunk:

  1. act:  ONE fp8 DoubleRow matmul per chunk contracts all 256 tokens:
           lhsT = [k0|k1] (128t x 2j x 128), rhs = [k0|k1|v0|v1] (x 256)
           -> pac[128, 256]: A0/A1 diag blocks of cols 0:128, ct0/ct1 diag
              blocks of cols 128:256 (junk off-diag).  4 chunks per pac.
  2. cast: per t-block (4 chunks), per pair, ONE activation moves the A/ct
           diag blocks into PERSISTENT pre-zeroed block-diag bf16 tiles
           (abct) - zeros off the diag keep the chain closed in block-diag.
  3. seed: matmul(pg = Id^T @ ctbd)   [start of PSUM accumulation group]
  4. grad: matmul(pg += Abd^T @ Wbd)  [stop]
     stt (DVE, ONE op): Wbd' = -s*pg + Wbd   (off-diag stays 0: 0*s+0)
  5. out:  matmul(pout = Wbd'(lhsT) @ qt[128 dpair x 256 tok]) - both heads
           in one 256-col matmul; emitted in chunk-PAIRS, one evac per pair.

The serial W-chain round trip (PE->DVE->PE ~1us) is hidden by round-robining
chunks across a 4-group window; acts/casts lead by LAG slots; outs trail.
PSUM budget (8 banks): pac 2x2 + pg 2 + pout 2.
GpSimd CANNOT access PSUM on TRN2, so casts go to Act and evacs to DVE/Act.

Device layouts (token t = c*256 + j*128 + p):
    kv (per group):  [128(p), 16(c), 2(j), 4(k0|k1|v0|v1), 64]  fp8 (v negated)
    qt (per group):  [128(hpair*64+d), 16(c), 256(t=j*128+p)]   bf16
    out (per group): [128(hpair*64+e), 16(c), 256(t)]           bf16
    W12bd: [hg, 128, 128] f32 block-diag(W_h0, W_h1); carried chain is bf16.
"""

import os
import sys

sys.path.insert(0, "/opt/trn_rl_repo")

import numpy as np

B, H, N, D = 16, 12, 4096, 64
N_ITERS = 16
M = N // N_ITERS  # 256 tokens per chunk
NCORES = 8
NB = B // NCORES  # batches per core
HG = H // 2  # head-pair groups per batch
SCALE = 1.0 / (M * D)
WAVE = 4  # chain interleave width (groups round-robined per chunk)
LAG = 14  # slots the act/cast stream leads the chain stream
CB = 4  # chunks per t-block (pac granularity)
USE_DR = True  # fp8 DoubleRow: one act matmul per chunk (else 2, j-accum)
DEFER_OUT_DMA = False

_CACHE = {}


def _split_excess_waits(nc):
    """walrus in this env accepts at most ONE sem wait per instruction
    (two on EventSemaphore); move excess waits onto EventSemaphore
    instructions inserted just before on the same engine."""
    import concourse.mybir as mybir

    n_ev = 0
    for f in nc.m.functions:
        for b in f.blocks:
            il = b.instructions
            idx = 0
            while idx < len(il):
                inst = il[idx]
                si = getattr(inst, "sync_info", None)
                if si is not None and len(si.on_wait) > 1:
                    waits = list(si.on_wait)
                    si.on_wait = [waits[0]]
                    extra = waits[1:]
                    for g in range(0, len(extra), 2):
                        n_ev += 1
                        ev = mybir.InstEventSemaphore(
                            name=f"EVSPLIT-{n_ev}",
                            engine=inst.engine,
                            ins=[],
                            outs=[],
                            sync_info=mybir.SyncInfo(
                                on_wait=extra[g : g + 2], on_update=[]
                            ),
                        )
                        nc.register_instruction(ev)
                        il.insert(idx, ev)
                        idx += 1
                idx += 1
    return n_ev


class _G:
    __slots__ = ("kv", "qt", "outsb", "wrep", "abct", "pac", "b", "gi")


def _build(nb=NB, hg=HG, n_iters=N_ITERS):
    import concourse.bass as bass
    import concourse.mybir as mybir
    from concourse.tile import TileContext

    f32 = mybir.dt.float32
    bf16 = mybir.dt.bfloat16
    fp8 = mybir.dt.float8e4
    Copy = mybir.ActivationFunctionType.Copy
    mult = mybir.AluOpType.mult
    add = mybir.AluOpType.add
    DR = mybir.MatmulPerfMode.DoubleRow

    ngroups = nb * hg  # 12
    nwaves = ngroups // WAVE  # 3
    slots_per_wave = WAVE * n_iters  # 64
    n_tb = n_iters // CB  # 4 t-blocks per group

    nc = bass.Bass()
    q_d = nc.declare_dram_parameter(
        "qt", [nb, hg, 128, n_iters * 256], bf16, isOutput=False
    )
    kv_d = nc.declare_dram_parameter(
        "kv", [nb, hg, 128, n_iters * 2 * 4 * D], fp8, isOutput=False
    )
    w_d = nc.declare_dram_parameter("W12bd", [128, hg * 128], f32, isOutput=False)
    id_d = nc.declare_dram_parameter("ident", [128, 128], bf16, isOutput=False)
    out_d = nc.declare_dram_parameter(
        "out", [nb, hg, 128, n_iters * 256], bf16, isOutput=True
    )

    with TileContext(nc) as tc:
        with (
            tc.tile_pool(name="singles", bufs=1) as singles,
            tc.tile_pool(name="kv", bufs=8) as kv_pool,
            tc.tile_pool(name="qt", bufs=6) as qt_pool,
            tc.tile_pool(name="osb", bufs=6) as osb_pool,
            tc.tile_pool(name="abct", bufs=8) as abct_pool,
            tc.tile_pool(name="wrp", bufs=20) as wrp_pool,
            tc.tile_pool(name="pac", bufs=2, space="PSUM") as pac_pool,
            tc.tile_pool(name="pg", bufs=2, space="PSUM") as pg_pool,
            tc.tile_pool(name="pout", bufs=2, space="PSUM") as pout_pool,
        ):
            ident = singles.tile([128, 128], bf16)
            nc.sync.dma_start(out=ident, in_=id_d[:, :])
            # hoist the lazy ACT_TABLE_LOAD off the first real cast's path
            dummy = singles.tile([128, 1], bf16)
            nc.scalar.activation(
                dummy, ident[:, 0:1], func=Copy, scale=1.0
            )
            winit = singles.tile([128, hg, 128], f32)
            winit_dma = [False]

            def load_winit():
                winit_dma[0] = True
                nc.sync.dma_start(
                    out=winit, in_=w_d.rearrange("p (g e) -> p g e", g=hg)
                )

            # persistent abct rotation: casts only ever write the diag
            # blocks, so the one-time memset zeros persist across reuses
            # (same logical tensors, manual rotation).  The memsets are
            # emitted from the schedule (after wave-0 DMA triggers) so they
            # don't block the GpSimd DMA queue at startup.
            abct_tiles = []
            for _i in range(8):
                abt = abct_pool.tile([128, CB, 2, 128], bf16, tag="abct")
                abct_tiles.append(abt)
            abct_ctr = [0]

            def memset_abct(i):
                nc.gpsimd.memset(abct_tiles[i], 0.0)

            glist = [None] * ngroups

            def ensure_group(gidx):
                if glist[gidx] is not None:
                    return
                g = _G()
                g.b, g.gi = divmod(gidx, hg)
                g.wrep = None
                g.abct = {}
                g.pac = None
                g.kv = None
                g.qt = None
                g.outsb = None
                glist[gidx] = g

            def init_wrep(gidx):
                # emitted strictly after load_winit so the copy waits on it
                ensure_group(gidx)
                g = glist[gidx]
                g.wrep = wrp_pool.tile([128, 128], bf16, tag="wrep")
                nc.vector.tensor_copy(g.wrep, winit[:, g.gi, :])

            def _q(gidx):
                # single queue: transfers fair-share DMA engines, so issue
                # order IS the priority order - keep inputs on one queue
                return nc.sync

            def kv_part(gidx, h, nh):
                # kv DMA in 1/nh fractions (latency vs per-transfer overhead)
                ensure_group(gidx)
                g = glist[gidx]
                if g.kv is None:
                    g.kv = kv_pool.tile(
                        [128, n_iters, 2, 4, D], fp8, tag="kv"
                    )
                hc = n_iters // nh
                w2 = hc * 2 * 4 * D
                _q(gidx).dma_start(
                    out=g.kv[:, h * hc : (h + 1) * hc, :, :, :],
                    in_=kv_d[g.b, g.gi, :, h * w2 : (h + 1) * w2].rearrange(
                        "p (c j s d) -> p c j s d", j=2, s=4, d=D
                    ),
                )

            def qt_part(gidx, h, nh):
                g = glist[gidx]
                if g.qt is None:
                    g.qt = qt_pool.tile([128, n_iters, 256], bf16, tag="qt")
                hc = n_iters // nh
                w2 = hc * 256
                _q(gidx).dma_start(
                    out=g.qt[:, h * hc : (h + 1) * hc, :],
                    in_=q_d[g.b, g.gi, :, h * w2 : (h + 1) * w2].rearrange(
                        "p (c t) -> p c t", t=256
                    ),
                )

            def emit_act(gidx, tb, u):
                # chunk c = CB*tb + u Gram matmul into pac[:, u, :, :]
                g = glist[gidx]
                c = CB * tb + u
                if u == 0:
                    g.pac = pac_pool.tile([128, CB, 2, 128], f32, tag="pac")
                if USE_DR:
                    nc.tensor.matmul(
                        g.pac[:, u, :, :],
                        lhsT=g.kv[:, c, :, 0:2, :],
                        rhs=g.kv[:, c, :, :, :],
                        start=True, stop=True,
                        perf_mode=DR,
                        skip_group_check=True,
                    )
                else:
                    for j in (0, 1):
                        nc.tensor.matmul(
                            g.pac[:, u, :, :],
                            lhsT=g.kv[:, c, j, 0:2, :],
                            rhs=g.kv[:, c, j, :, :],
                            start=(j == 0), stop=(j == 1),
                            skip_group_check=True,
                        )

            def emit_cast(gidx, tb):
                # A/ct diag blocks -> block-diag bf16 (abct off-diag stays 0)
                g = glist[gidx]
                ab = abct_tiles[abct_ctr[0] % len(abct_tiles)]
                on_dve = abct_ctr[0] % 2 == 0
                abct_ctr[0] += 1
                nc.scalar.activation(
                    ab[0:64, :, :, 0:64], g.pac[0:64, :, :, 0:64],
                    func=Copy, scale=1.0,
                )
                if on_dve:
                    nc.vector.tensor_copy(
                        ab[64:128, :, :, 64:128], g.pac[64:128, :, :, 64:128]
                    )
                else:
                    nc.scalar.activation(
                        ab[64:128, :, :, 64:128], g.pac[64:128, :, :, 64:128],
                        func=Copy, scale=1.0,
                    )
                g.abct[tb] = ab
                g.pac = None

            def chain_seed(g, c):
                tb, u = divmod(c, CB)
                ab = g.abct[tb]
                pg = pg_pool.tile([128, 512], f32, tag="pg")
                nc.tensor.matmul(
                    pg[:, 0:128],
                    lhsT=ident[:, :],
                    rhs=ab[:, u, 1, :],
                    start=True, stop=False, skip_group_check=True,
                )
                return pg

            def chain_grad(g, c, pg):
                tb, u = divmod(c, CB)
                ab = g.abct[tb]
                nc.tensor.matmul(
                    pg[:, 0:128],
                    lhsT=ab[:, u, 0, :],
                    rhs=g.wrep[:, :],
                    start=False, stop=True, skip_group_check=True,
                )
                wnew = wrp_pool.tile([128, 128], bf16, tag="wrep")
                nc.vector.scalar_tensor_tensor(
                    wnew, pg[:, 0:128], -SCALE, g.wrep,
                    op0=mult, op1=add,
                )
                g.wrep = wnew
                if u == CB - 1:
                    del g.abct[tb]

            def emit_out_mm(gidx, c, wrep, po, slot_idx):
                g = glist[gidx]
                nc.tensor.matmul(
                    po[:, slot_idx, :], lhsT=wrep[:, :], rhs=g.qt[:, c, :],
                    start=True, stop=True, skip_group_check=True,
                )

            def emit_evac(gidx, c0, po, evac_on_act):
                g = glist[gidx]
                if g.outsb is None:
                    g.outsb = osb_pool.tile(
                        [128, n_iters, 256], bf16, tag="osb"
                    )
                dst = g.outsb[:, c0 : c0 + 2, :]
                if evac_on_act:
                    nc.scalar.activation(dst, po, func=Copy, scale=1.0)
                else:
                    nc.vector.tensor_copy(dst, po)
                half = n_iters * 256 // 2
                quart = half // 2
                last_wave = gidx >= ngroups - WAVE
                if last_wave:
                    # last wave: drain in quarters over the idle queues so
                    # the epilogue tail is short
                    oq = (nc.gpsimd, nc.sync, nc.scalar, nc.gpsimd)[gidx % 4]
                    if c0 + 1 == n_iters // 2 - 1:
                        oq.dma_start(
                            out=out_d[g.b, g.gi, :, 0:half],
                            in_=g.outsb[:, 0 : n_iters // 2, :],
                        )
                    elif c0 + 1 == 3 * n_iters // 4 - 1:
                        oq.dma_start(
                            out=out_d[g.b, g.gi, :, half : half + quart],
                            in_=g.outsb[:, n_iters // 2 : 3 * n_iters // 4, :],
                        )
                    elif c0 + 1 == n_iters - 1:
                        oq.dma_start(
                            out=out_d[g.b, g.gi, :, half + quart :],
                            in_=g.outsb[:, 3 * n_iters // 4 :, :],
                        )
                        g.outsb = None
                        g.qt = None
                        g.kv = None
                    return
                if c0 + 1 == n_iters // 2 - 1:
                    nc.gpsimd.dma_start(
                        out=out_d[g.b, g.gi, :, 0:half],
                        in_=g.outsb[:, 0 : n_iters // 2, :],
                    )
                elif c0 + 1 == n_iters - 1:
                    if DEFER_OUT_DMA:
                        # defer the 2nd-half DMA past the wave boundary so
                        # it doesn't steal HBM from the next wave's kv/qt
                        defer_out.append((gidx, g.outsb))
                    else:
                        nc.gpsimd.dma_start(
                            out=out_d[g.b, g.gi, :, half : 2 * half],
                            in_=g.outsb[:, n_iters // 2 : n_iters, :],
                        )
                    g.outsb = None
                    g.qt = None
                    g.kv = None

            # ---------------- schedule -----------------------------------
            # chain slot s (0..191): wave w = s//64, r = s%64, c = r//WAVE,
            #   gp = r%WAVE, group g = w*WAVE+gp.
            # act item (g, tb): 4 DR matmuls at slots w*64+16*tb+gp-LAG ...
            #   +3, cast at +4.
            # group kv DMA one wave ahead (spread), qt half a wave ahead.
            events = {}

            def at(slot, fn, *args):
                events.setdefault(slot, []).append((fn, args))

            n_slots = nwaves * slots_per_wave
            for w in range(nwaves):
                for gp in range(WAVE):
                    gidx = w * WAVE + gp
                    if w == 0:
                        # wave 0: kv quarters, just-in-time priority order
                        # (first chain slots gate on the least DMA bytes)
                        at(-50 + gp, kv_part, gidx, 0, 4)
                        at(-45 + gp, kv_part, gidx, 1, 4)
                        at(-40 + gp, qt_part, gidx, 0, 2)
                        at(-36 + gp, kv_part, gidx, 2, 4)
                        at(-32 + gp, qt_part, gidx, 1, 2)
                        at(-28 + gp, kv_part, gidx, 3, 4)
                    else:
                        # kv one wave early (highest priority after wave 0);
                        # qt later - first needed only ~6 slots into a wave
                        at(w * 64 - 88 + 4 * gp, kv_part, gidx, 0, 1)
                        at(w * 64 - 71 + 4 * gp, qt_part, gidx, 0, 1)
                    at(w * 64 - 43 + gp if w == 0 else w * 64 - 86 + 4 * gp,
                       init_wrep, gidx)
                    for tb in range(n_tb):
                        t0 = w * 64 + 16 * tb + 4 * gp - LAG
                        for u in range(CB):
                            at(t0 + u, emit_act, gidx, tb, u)
                        at(t0 + CB, emit_cast, gidx, tb)
            at(-46, load_winit)
            for i in range(8):
                at(-58 + i, memset_abct, i)

            # pending out-pairs: (gidx, c0, w0, w1)
            pend = []
            po_open = [None]
            prev_w = [None] * ngroups
            evac_flip = [0]
            defer_out = []

            def flush_deferred_outs():
                half = n_iters * 256 // 2
                while defer_out:
                    gi2, osb = defer_out.pop(0)
                    b2, g2 = divmod(gi2, hg)
                    nc.gpsimd.dma_start(
                        out=out_d[b2, g2, :, half : 2 * half],
                        in_=osb[:, n_iters // 2 : n_iters, :],
                    )

            for w in range(1, nwaves):
                at(w * 64 + 8, flush_deferred_outs)

            lo = min(events)
            for s in range(lo, n_slots + 3):
                # slot order: chain-critical ops first (outs, seed, grad,
                # stt, evac), then act/cast/DMA events - so the engine
                # queues see latency-critical work ahead of slack work,
                # and no two consecutive matmuls share a PSUM bank.
                # slot order: seed+grad FIRST (stt fires earliest, so the
                # 2-slot pg rotation incurs no WAR wait), then ONE out-half
                # (separates consecutive slots' pout/pg banks), then evac.
                in_chain = 0 <= s < n_slots
                if in_chain:
                    w, r = divmod(s, slots_per_wave)
                    c, gp = divmod(r, WAVE)
                    gidx = w * WAVE + gp
                    g = glist[gidx]
                    pg = chain_seed(g, c)
                    chain_grad(g, c, pg)
                    if c % 2 == 1:
                        pend.append((gidx, c - 1, prev_w[gidx], g.wrep))
                    else:
                        prev_w[gidx] = g.wrep
                if po_open[0] is not None:
                    gq, c0q, w1q, po, on_act = po_open[0]
                    po_open[0] = None
                    emit_out_mm(gq, c0q + 1, w1q, po, 1)
                    emit_evac(gq, c0q, po, on_act)
                elif pend and s >= 6 and (s % slots_per_wave >= 8
                                          or s >= n_slots or len(pend) > 5):
                    gq, c0q, w0q, w1q = pend.pop(0)
                    evac_flip[0] = (evac_flip[0] + 1) % 3
                    on_act = evac_flip[0] != 0  # 2/3 Act, 1/3 DVE
                    po = pout_pool.tile([128, 2, 256], f32, tag="po")
                    emit_out_mm(gq, c0q, w0q, po, 0)
                    po_open[0] = (gq, c0q, w1q, po, on_act)
                for fn, args in events.get(s, ()):
                    fn(*args)
            while pend or po_open[0] is not None:
                if po_open[0] is not None:
                    gq, c0q, w1q, po, on_act = po_open[0]
                    po_open[0] = None
                    emit_out_mm(gq, c0q + 1, w1q, po, 1)
                    emit_evac(gq, c0q, po, on_act)
                    continue
                gq, c0q, w0q, w1q = pend.pop(0)
                evac_flip[0] = (evac_flip[0] + 1) % 3
                po = pout_pool.tile([128, 2, 256], f32, tag="po")
                emit_out_mm(gq, c0q, w0q, po, 0)
                po_open[0] = (gq, c0q, w1q, po, True)

    _split_excess_waits(nc)
    return nc


def _get_nc():
    if "nc" not in _CACHE:
        _CACHE["nc"] = _build()
    return _CACHE["nc"]


def _host_prep(q, k, v):
    """Host re-layout (token t = c*256 + j*128 + p)."""
    import ml_dtypes

    bf = ml_dtypes.bfloat16
    f8 = ml_dtypes.float8_e4m3
    Bq, Hq, Nq, Dq = q.shape
    hg = Hq // 2
    ni = Nq // 256
    # kv: [b, g, p, c, j, (k0|k1|v0|v1), d]
    k7 = k.reshape(Bq, hg, 2, ni, 2, 128, Dq)
    v7 = (-v).reshape(Bq, hg, 2, ni, 2, 128, Dq)
    kv = np.stack(
        [k7[:, :, 0], k7[:, :, 1], v7[:, :, 0], v7[:, :, 1]], axis=5
    )  # [b, g, c, j, p, 4, d]
    kv = np.ascontiguousarray(
        kv.transpose(0, 1, 4, 2, 3, 5, 6).reshape(Bq, hg, 128, ni * 2 * 4 * Dq)
    ).astype(f8)
    # qt: [b, g, hpair*64+d, c, t]
    q6 = q.reshape(Bq, hg, 2, ni, 256, Dq)
    qt = np.ascontiguousarray(
        q6.transpose(0, 1, 2, 5, 3, 4).reshape(Bq, hg, 128, ni * 256)
    ).astype(bf)
    return kv, qt


def _host_unshuffle(out_host):
    """[b, g, hpair*64+e, c*256+t] bf16 -> (B, N, H*64) f32."""
    Bq, hgq, _, w = out_host.shape
    ni = w // 256
    o6 = np.asarray(out_host, dtype=np.float32).reshape(
        Bq, hgq, 2, 64, ni, 256
    )
    # [b, g, hp, e, c, t] -> [b, c, t, g, hp, e]
    return np.ascontiguousarray(
        o6.transpose(0, 4, 5, 1, 2, 3).reshape(Bq, ni * 256, hgq * 2 * 64)
    )


def kernel(q, k, v, W_init, training=0, return_aux=0, **_unused):
    import ml_dtypes
    from concourse.bass_utils import run_bass_kernel_spmd

    q = np.asarray(q, dtype=np.float32)
    k = np.asarray(k, dtype=np.float32)
    v = np.asarray(v, dtype=np.float32)
    W_init = np.ascontiguousarray(np.asarray(W_init, dtype=np.float32))

    kv, qt = _host_prep(q, k, v)
    wbd = np.zeros((HG, 128, 128), dtype=np.float32)
    wbd[:, 0:64, 0:64] = W_init[0::2]
    wbd[:, 64:128, 64:128] = W_init[1::2]
    wbd = np.ascontiguousarray(
        wbd.transpose(1, 0, 2).reshape(128, HG * 128)
    )
    ident = np.eye(128, dtype=ml_dtypes.bfloat16)

    nc = _get_nc()
    in_maps = []
    for i in range(NCORES):
        sl = slice(i * NB, (i + 1) * NB)
        in_maps.append(
            {"qt": qt[sl], "kv": kv[sl], "W12bd": wbd, "ident": ident}
        )

    trace = bool(int(os.environ.get("BASS_KERNEL_TRACE", "0")))
    res = run_bass_kernel_spmd(
        nc, in_maps, core_ids=list(range(NCORES)), trace=trace
    )
    _CACHE["last_results"] = res
    out_host = np.concatenate(
        [np.asarray(res.results[i]["out"]) for i in range(NCORES)], axis=0
    )
    return _host_unshuffle(out_host)


if __name__ == "__main__":
    rng = np.random.default_rng(0)
    q = rng.standard_normal((B, H, N, D), dtype=np.float32)
    k = rng.standard_normal((B, H, N, D), dtype=np.float32)
    v = rng.standard_normal((B, H, N, D), dtype=np.float32)
    W = (rng.standard_normal((H, D, D)) * D**-0.5).astype(np.float32)
    out = kernel(q, k, v, W)
    print("kernel ran, out shape:", out.shape)
